# revision 33
# baseline (speedup 1.0000x reference)
"""Multi-head attention (RMSNorm-QK + RoPE) Trainium2 Bass kernel — v3.

Sharding: 8 cores = 4 batches x 2 head-groups (6 heads each).
Host sums the two partial y's per batch and adds proj bias.

v3 design (vs v2):
  - QK slabs shrink to 2 banks (2 fills/chunk, 192 units): frees 2 PSUM
    banks for a dedicated piece pool, so deferred-q GEMMs / broadcast /
    projection matmuls no longer stall the QK->exp->PV rotation.
  - QK fill pairs are (kt,hh0)+(kt,hh1): every pair runs concurrently in
    disjoint PE row groups.
  - Attention units for (qc0,pp0) are interleaved into phase-1 tile
    production, so the scalar-engine exp stream starts at ~25us.
  - Softmax denominators are gathered onto partitions 0..5 by DVE row
    copies, ln/exp runs 6-lane (was 1-lane), reciprocal rows copied back
    to partition 64 for the broadcast matmuls.
  - vaug copy and the phase-1 PSUM->SBUF q/k copy run on DVE during the
    interleaved phase to keep ACT pure-exp.
"""

import sys

for _p in ("/opt/trn_rl_repo", "/root/.axon_site/_ro/trn_rl_repo"):
    if _p not in sys.path:
        sys.path.insert(0, _p)

import numpy as np
import ml_dtypes

import bass_rust
import concourse.bass as bass
import concourse.mybir as mybir
import concourse.tile as tile
from concourse.bass_utils import run_bass_kernel_spmd

# Problem constants (hardcoded per contract)
B, N, D = 4, 2048, 768
H, HD = 12, 64
HPC = 6              # heads per core
NT = N // 128        # 16 seq tiles
EPS = 1e-6
THETA = 10000.0
SCALE = HD ** -0.5   # 0.125

F32 = mybir.dt.float32
F32R = mybir.dt.float32r
BF16 = mybir.dt.bfloat16

KERNEL_TRACE = False
_CACHE = {}


# ---------------------------------------------------------------- wait split
_ctr = [0]


def _mk_nop(engine, waits=None, updates=None):
    _ctr[0] += 1
    si = mybir.SyncInfo(on_wait=waits or [], on_update=updates or [])
    return bass_rust.InstNoOp(
        name=f"I-waitfix-{_ctr[0]}", engine=engine, ins=[], outs=[], sync_info=si
    )


def split_multi_waits(nc):
    """This walrus build accepts only ONE sync wait/update per instruction;
    hoist extras onto adjacent same-engine NoOp carriers."""
    for fn in nc.m.functions:
        for bb in fn.blocks:
            insts = bb.instructions
            out = []
            changed = False
            for inst in insts:
                si = inst.sync_info
                if si is None:
                    out.append(inst)
                    continue
                waits = list(si.on_wait or [])
                updates = list(si.on_update or [])
                pre, post = [], []
                if len(waits) > 1:
                    for w in waits[:-1]:
                        pre.append(_mk_nop(inst.engine, waits=[w]))
                    si.on_wait = [waits[-1]]
                    changed = True
                if len(updates) > 1:
                    if inst.opcode == "DMACopy":
                        raise RuntimeError(
                            f"DMACopy {inst.name} has {len(updates)} updates"
                        )
                    for u in updates[1:]:
                        post.append(_mk_nop(inst.engine, updates=[u]))
                    si.on_update = [updates[0]]
                    changed = True
                out.extend(pre)
                out.append(inst)
                out.extend(post)
            if changed:
                insts[:] = out
    return nc


# ---------------------------------------------------------------- host utils
def _rope_tables(norm_w: np.ndarray):
    """cosw[n,d] = cos[n,d]*w[d];  sinw folds the rotate-half sign+swap of w:
    q' = qn*cosw + shuffle32(qn)*sinw  (shuffle32 = swap halves, no negation)."""
    inv_freq = 1.0 / (THETA ** (np.arange(0, HD, 2, dtype=np.float32) / HD))
    t = np.arange(N, dtype=np.float32)
    freqs = np.einsum("i,j->ij", t, inv_freq).astype(np.float32)
    emb = np.concatenate([freqs, freqs], axis=-1)  # [N, HD]
    cos = np.cos(emb).astype(np.float32)
    sin = np.sin(emb).astype(np.float32)
    w = norm_w.astype(np.float32)
    cosw = cos * w[None, :]
    sinw = np.empty_like(sin)
    h = HD // 2
    sinw[:, :h] = -sin[:, :h] * w[None, h:]
    sinw[:, h:] = sin[:, h:] * w[None, :h]
    return cosw, sinw


# ---------------------------------------------------------------- bass build
def build_nc(use_bias: bool):
    FC = 7 if use_bias else 6  # feature chunks of 128 (7th = bias row)
    nc = bass.Bass()

    xt_d = nc.dram_tensor("xt", [FC * 128, N], BF16, kind="ExternalInput")
    wq_d = nc.dram_tensor("wq", [FC * 128, HPC * HD], BF16, kind="ExternalInput")
    wk_d = nc.dram_tensor("wk", [FC * 128, HPC * HD], BF16, kind="ExternalInput")
    wv_d = nc.dram_tensor("wv", [FC * 128, HPC * HD], BF16, kind="ExternalInput")
    wo_d = nc.dram_tensor("wo", [HPC * HD, D], BF16, kind="ExternalInput")
    # rope tables: [:, 0, :] = q variant, [:, 1, :] = k variant (norm_w folded)
    cos_d = nc.dram_tensor("cost", [N, 2 * HD], BF16, kind="ExternalInput")
    sin_d = nc.dram_tensor("sint", [N, 2 * HD], BF16, kind="ExternalInput")
    y_d = nc.dram_tensor("y", [N, D], F32, kind="ExternalOutput")

    NQC = 4              # query chunks of 512
    NCI = NT             # 16 chunks of (kt, hh0)+(kt, hh1) per (qc, pp)

    with tile.TileContext(nc) as tc:
        with (
            tc.tile_pool(name="const", bufs=1) as constp,
            tc.tile_pool(name="wts", bufs=1) as wts,
            tc.tile_pool(name="persist", bufs=1) as persist,
            tc.tile_pool(name="rope", bufs=2) as rope,
            tc.tile_pool(name="ptp", bufs=3) as ptp,
            tc.tile_pool(name="otunp", bufs=2) as otunp,
            tc.tile_pool(name="otqp", bufs=2) as otqp,
            tc.tile_pool(name="denp", bufs=2) as denp,
            tc.tile_pool(name="yout", bufs=2) as yout,
            tc.tile_pool(name="work", bufs=2, space="PSUM") as workp,
            tc.tile_pool(name="gemp", bufs=1, space="PSUM") as gemp,
            tc.tile_pool(name="otp", bufs=2, space="PSUM") as otp,
        ):
            # ---- constants / weights
            ones_sb = constp.tile([128, 64], BF16)
            nc.vector.memset(ones_sb[:, :], 1.0)
            eps_t = constp.tile([128, 1], F32)
            nc.vector.memset(eps_t[:, :], EPS)
            zero_t = constp.tile([128, 1], F32)
            nc.vector.memset(zero_t[:, :], 0.0)

            # x stays SBUF-resident: 4 quarter-DMAs (one trigger each; each
            # spreads over all 16 hw queues).  First GEMM needs only quarter 0
            # plus wk/wv — ~5 triggers instead of 25.
            xt_full = persist.tile([128, FC, N], BF16, tag="xtf")
            xt_dr = xt_d.rearrange("(c p) n -> p c n", p=128)

            def load_xq(q):
                nc.sync.dma_start(
                    xt_full[:, :, q * 512:(q + 1) * 512],
                    xt_dr[:, :, q * 512:(q + 1) * 512],
                )

            load_xq(0)
            w_sbs = []
            for wd, nm in ((wk_d, "wk"), (wv_d, "wv"), (wq_d, "wq")):
                wsb = wts.tile([128, FC, HPC * HD], BF16, tag=nm, name=nm)
                nc.sync.dma_start(
                    wsb[:, :, :], wd.rearrange("(c p) n -> p c n", p=128))
                w_sbs.append(wsb)
            w_sbs = [w_sbs[2], w_sbs[0], w_sbs[1]]  # back to q, k, v order
            # rope tables (bf16 host-side)
            cos_sb = constp.tile([128, NT, 2, HD], BF16, tag="cos")
            nc.sync.dma_start(
                cos_sb[:, :, :, :],
                cos_d.rearrange("(t p) (a d) -> p t a d", p=128, a=2),
            )
            sin_sb = constp.tile([128, NT, 2, HD], BF16, tag="sin")
            nc.sync.dma_start(
                sin_sb[:, :, :, :],
                sin_d.rearrange("(t p) (a d) -> p t a d", p=128, a=2),
            )
            wo_sb = wts.tile([128, 3, D], BF16, tag="wo")
            nc.sync.dma_start(wo_sb[:, :, :], wo_d.rearrange("(c p) n -> p c n", p=128))
            for _q in range(1, 4):
                load_xq(_q)

            qt_sb = persist.tile([128, 3, N], BF16, tag="qt")
            kt_sb = persist.tile([128, 3, N], BF16, tag="kt")
            vaug = persist.tile([128, NT, HPC, 65], BF16, tag="vaug")
            nc.vector.memset(vaug[:, :, :, 64:65], 1.0)

            # ================= phase 1 helpers =================
            NQ_EARLY = 4

            def ph1_stage_b(p):
                # p = (i, na, v0, ss, c2); variants v0..v0+na-1 (0=q, 1=k)
                i, na, v0, ss, c2 = p
                lg = rope.tile([128, na, HPC], F32, tag="lg")
                nc.scalar.activation(lg[:, :, :], ss[:, :, :],
                                     mybir.ActivationFunctionType.Ln,
                                     bias=eps_t[:, :], scale=1.0 / HD)
                rs = rope.tile([128, na, HPC], BF16, tag="rs")
                nc.scalar.activation(rs[:, :, :], lg[:, :, :],
                                     mybir.ActivationFunctionType.Exp,
                                     bias=zero_t[:, :], scale=-0.5)
                ro = rope.tile([128, na, HPC, HD], BF16, tag="ro")
                nc.vector.tensor_mul(
                    ro[:, :, :, :], c2[:, :, :, :],
                    rs[:, :, :, None].to_broadcast((128, na, HPC, HD)),
                )
                rof = ro.rearrange("p a h d -> p (a h d)")
                for j in range(na):
                    dst = qt_sb if v0 + j == 0 else kt_sb
                    nc.sync.dma_start_transpose(
                        dst[:, 0:3, i * 128:(i + 1) * 128],
                        rof[:, j * 384:(j + 1) * 384])

            def norm_rope(i, ps_ap, na, v0, path):
                """ps_ap: PSUM [128, na, 384] raw q/k; returns (ss, c2).
                path: 'act' = copy+square on ACT, 'dve' = both on DVE,
                'mix' = copy on DVE + square on ACT."""
                src = ps_ap.rearrange("p a (h d) -> p a h d", h=HPC)
                qk = rope.tile([128, na, HPC, HD], BF16, tag="qk")
                sq = rope.tile([128, na, HPC, HD], BF16, tag="sq")
                if path == "act":
                    nc.scalar.copy(qk[:, :, :, :], src)
                    nc.scalar.activation(sq[:, :, :, :], src,
                                         mybir.ActivationFunctionType.Square,
                                         bias=zero_t[:, :])
                elif path == "mix":
                    nc.vector.tensor_copy(qk[:, :, :, :], src)
                    nc.scalar.activation(sq[:, :, :, :], src,
                                         mybir.ActivationFunctionType.Square,
                                         bias=zero_t[:, :])
                else:
                    nc.vector.tensor_copy(qk[:, :, :, :], src)
                    nc.vector.tensor_mul(sq[:, :, :, :], qk[:, :, :, :],
                                         qk[:, :, :, :])
                ss = rope.tile([128, na, HPC], F32, tag="ss")
                nc.vector.reduce_sum(ss[:, :, :], sq[:, :, :, :],
                                     axis=mybir.AxisListType.X)
                cosb = cos_sb[:, i, v0:v0 + na, None, :].to_broadcast(
                    (128, na, HPC, HD))
                sinb = sin_sb[:, i, v0:v0 + na, None, :]
                a = rope.tile([128, na, HPC, HD], BF16, tag="a")
                nc.vector.tensor_mul(a[:, :, :, :], qk[:, :, :, :], cosb)
                bt = rope.tile([128, na, HPC, HD], BF16, tag="bt")
                h = HD // 2
                nc.vector.tensor_mul(
                    bt[:, :, :, 0:h], qk[:, :, :, h:HD],
                    sinb[:, :, :, 0:h].to_broadcast((128, na, HPC, h)))
                nc.vector.tensor_mul(
                    bt[:, :, :, h:HD], qk[:, :, :, 0:h],
                    sinb[:, :, :, h:HD].to_broadcast((128, na, HPC, h)))
                c2 = rope.tile([128, na, HPC, HD], BF16, tag="c2")
                nc.vector.tensor_add(c2[:, :, :, :], a[:, :, :, :],
                                     bt[:, :, :, :])
                return ss, c2

            ph1_pend = []

            def flush_ph1():
                for pd in ph1_pend:
                    ph1_stage_b(pd)
                ph1_pend.clear()

            def tile_gemm(i, with_q, kv_pool):
                """K+V GEMM (+Q first, for early tiles) for seq tile i."""
                xs = xt_full[:, :, i * 128:(i + 1) * 128]
                psq = None
                if with_q:
                    # Q leads: qc0 units can't start until all four early q
                    # transposes land, so q's chain goes first everywhere.
                    psq = workp.tile([128, 2, 512], F32, tag="work",
                                     name=f"q{i}")
                    for c in range(FC):
                        nc.tensor.matmul(
                            psq[:, 0, 0:384], xs[:, c, :], w_sbs[0][:, c, :],
                            start=(c == 0), stop=(c == FC - 1),
                        )
                ps = kv_pool.tile([128, 2, 512], F32,
                                  tag="gp" if kv_pool is gemp else "work",
                                  name=f"kv{i}")
                for c in range(FC):
                    for t, wsb in enumerate((w_sbs[1], w_sbs[2])):
                        nc.tensor.matmul(
                            ps[:, t, 0:384], xs[:, c, :], wsb[:, c, :],
                            start=(c == 0), stop=(c == FC - 1),
                        )
                return ps, psq

            def tile_chain(i, ps, psq, path):
                pend_local = []
                if psq is not None:
                    ssq, c2q = norm_rope(i, psq[:, 0:1, 0:384], 1, 0, path)
                    if i == NQ_EARLY - 1:
                        # last early tile's q stage_b runs eagerly: the qc0
                        # units gate on this one transpose landing.
                        ph1_stage_b((i, 1, 0, ssq, c2q))
                    else:
                        pend_local.append((i, 1, 0, ssq, c2q))
                # V copy to vaug (DVE keeps ACT free for exp)
                nc.vector.tensor_copy(
                    vaug[:, i, :, 0:64],
                    ps[:, 1, 0:384].rearrange("p (h d) -> p h d", h=HPC),
                )
                ssk, c2k = norm_rope(i, ps[:, 0:1, 0:384], 1, 1, path)
                pend_local.append((i, 1, 1, ssk, c2k))
                ph1_pend.extend(pend_local)

            # ================= attention unit machinery =================
            cur_ots = {}
            otun_by_qc = {}
            den_by_qc = {}       # qc -> (den6 [6,512] f32, rec6 [6,512] bf16)
            rec_by_qc = {}       # qc -> rec64 [65, HPC, 512] bf16
            otq_by_qc = {}
            finish_pieces = []
            pend = [None]        # (qc, pp, ci, pt)

            def emit_den6(qc):
                # Gather the 6 den rows (partition 64 of otun) onto partitions
                # 0..5 via tiny SBUF->SBUF DMAs (engines can't address base
                # partitions outside {0,32,64,96}; DMA can), ln/exp 6-lane,
                # then scatter the reciprocal rows back to partition 64.
                otun_all = otun_by_qc[qc]
                den6i = denp.tile([6, 512], BF16, tag="den6i",
                                  name=f"den6i_{qc}")
                lnt = denp.tile([6, 512], F32, tag="lnt", name=f"lnt_{qc}")
                rec6 = denp.tile([6, 512], BF16, tag="rec6", name=f"rec6_{qc}")
                rec64 = rec_by_qc.setdefault(
                    qc, denp.tile([65, HPC, 512], BF16, tag="rec64",
                                  name=f"rec64_{qc}"))
                for hloc in range(HPC):
                    nc.sync.dma_start(den6i[hloc:hloc + 1, :],
                                      otun_all[64:65, hloc, :])
                nc.scalar.activation(lnt[0:6, :], den6i[0:6, :],
                                     mybir.ActivationFunctionType.Ln,
                                     bias=zero_t[0:6, :], scale=1.0)
                nc.scalar.activation(rec6[0:6, :], lnt[0:6, :],
                                     mybir.ActivationFunctionType.Exp,
                                     bias=zero_t[0:6, :], scale=-1.0)
                for hloc in range(HPC):
                    nc.sync.dma_start(rec64[64:65, hloc, :],
                                      rec6[hloc:hloc + 1, :])

            def emit_den_tail(qc, h0, h1):
                # Tail path (last qc): 1-lane ln/exp directly on the otun den
                # rows at partition 64 — no DMA latency in the critical tail.
                otun_all = otun_by_qc[qc]
                lg3 = denp.tile([65, 2, 512], F32, tag="lg3", name="lg3")
                rec64 = rec_by_qc.setdefault(
                    qc, denp.tile([65, HPC, 512], BF16, tag="rec64",
                                  name=f"rec64_{qc}"))
                nc.scalar.activation(lg3[64:65, :, :],
                                     otun_all[64:65, h0:h1, :],
                                     mybir.ActivationFunctionType.Ln,
                                     bias=zero_t[64:65, :], scale=1.0)
                nc.scalar.activation(rec64[64:65, h0:h1, :], lg3[64:65, :, :],
                                     mybir.ActivationFunctionType.Exp,
                                     bias=zero_t[64:65, :], scale=-1.0)

            def norm_piece(qc, h0, h1):
                def fn():
                    otun_all = otun_by_qc[qc]
                    rec64 = rec_by_qc[qc]
                    if qc not in otq_by_qc:
                        otq_by_qc[qc] = otqp.tile([128, 3, 512], BF16,
                                                  tag="otq", name=f"otq{qc}")
                    otq = otq_by_qc[qc]
                    bcw = gemp.tile([128, 2, 512], F32, tag="gp",
                                    name="bcw")
                    for j, hloc in enumerate(range(h0, h1)):
                        pp_, hh_ = hloc // 2, hloc % 2
                        nc.tensor.matmul(bcw[0:64, j, :],
                                         ones_sb[64:65, :],
                                         rec64[64:65, hloc, :],
                                         start=True, stop=True)
                        nc.vector.tensor_mul(
                            otq[hh_ * 64:(hh_ + 1) * 64, pp_, :],
                            otun_all[0:64, hloc, :],
                            bcw[0:64, j, :],
                        )
                return fn

            def proj_piece(qc, t0, t1, pool=None, tail=False):
                def fn():
                    pl = pool if pool is not None else gemp
                    otq = otq_by_qc[qc]
                    for qt4 in range(t0, t1):
                        q0 = qc * 512 + qt4 * 128
                        yps = pl.tile([128, 2, 512], F32,
                                      tag="gp" if pl is gemp else "work",
                                      name="yps")
                        for c in range(3):
                            nc.tensor.matmul(
                                yps[:, 0, :],
                                otq[:, c, qt4 * 128:(qt4 + 1) * 128],
                                wo_sb[:, c, 0:512],
                                start=(c == 0), stop=(c == 2),
                            )
                        for c in range(3):
                            nc.tensor.matmul(
                                yps[:, 1, 0:256],
                                otq[:, c, qt4 * 128:(qt4 + 1) * 128],
                                wo_sb[:, c, 512:768],
                                start=(c == 0), stop=(c == 2),
                            )
                        ysb = yout.tile([128, D], F32, tag="ysb")
                        if tail:
                            # split across engines — ACT is idle in the tail
                            nc.scalar.copy(ysb[:, 0:512], yps[:, 0, :])
                        else:
                            nc.vector.tensor_copy(ysb[:, 0:512], yps[:, 0, :])
                        nc.vector.tensor_copy(ysb[:, 512:768], yps[:, 1, 0:256])
                        nc.sync.dma_start(y_d[q0:q0 + 128, :], ysb[:, :])
                return fn

            # deferred Q work for tiles 4-15, as pieces (x is SBUF-resident)
            q_state = {}

            def q_piece_a(i):
                def fn():
                    psq = gemp.tile([128, 2, 512], F32, tag="gp",
                                    name=f"psq{i}")
                    for c in range(FC):
                        nc.tensor.matmul(
                            psq[:, 0, 0:384], xt_full[:, c, i * 128:(i + 1) * 128],
                            w_sbs[0][:, c, :],
                            start=(c == 0), stop=(c == FC - 1),
                        )
                    # DVE-path norm stats (ACT is saturated by exp)
                    q_state[i] = norm_rope(i, psq[:, 0:1, 0:384], 1, 0, "dve")
                return fn

            def q_piece_b(i):
                def fn():
                    ss, c2 = q_state.pop(i)
                    ph1_stage_b((i, 1, 0, ss, c2))
                return fn

            def flush_pv(p):
                qc, pp, ci, pt = p
                key = (qc, pp)
                if key not in cur_ots:
                    cur_ots[key] = [
                        otp.tile([128, 512], F32, tag="ot", name=f"ots{hh}")
                        for hh in range(2)
                    ]
                ots = cur_ots[key]
                for hh in range(2):
                    nc.tensor.matmul(
                        ots[hh][0:65, :],
                        vaug[:, ci, pp * 2 + hh, :],
                        pt[:, hh, :],
                        start=(ci == 0), stop=(ci == NCI - 1),
                    )
                if ci == NCI - 1:
                    if qc not in otun_by_qc:
                        otun_by_qc[qc] = otunp.tile(
                            [65, HPC, 512], BF16, tag="otun", name=f"otun{qc}")
                    otun_all = otun_by_qc[qc]
                    for hh in range(2):
                        nc.vector.tensor_copy(
                            otun_all[0:65, pp * 2 + hh, :], ots[hh][0:65, :])
                    del cur_ots[key]
                    if qc == 3:
                        # Last qc: den + normalize per pp so the final
                        # projection isn't one serial tail.
                        emit_den_tail(3, pp * 2, pp * 2 + 2)
                        norm_piece(3, pp * 2, pp * 2 + 2)()
                    elif pp == 2:
                        emit_den6(qc)
                elif ci == 4 and pp == 0 and qc > 0:
                    # qc-1's normalize/projection pieces: enqueue a few units
                    # into this qc (den6 DMA chain has landed by now) so they
                    # spread over ~44 piece slots instead of ~28.
                    finish_pieces.append((500, norm_piece(qc - 1, 0, 2)))
                    finish_pieces.append((500, norm_piece(qc - 1, 2, 4)))
                    finish_pieces.append((500, norm_piece(qc - 1, 4, 6)))
                    for t in range(4):
                        finish_pieces.append(
                            (1900, proj_piece(qc - 1, t, t + 1)))

            unit_no = [0]
            pe_cap = [0]

            def emit_unit(qc, pp, ci, pop_piece=True):
                slab = workp.tile([128, 2, 512], F32, tag="work", name="slab")
                for hh in range(2):
                    nc.tensor.matmul(
                        slab[:, hh, :],
                        kt_sb[hh * 64:(hh + 1) * 64, pp, ci * 128:(ci + 1) * 128],
                        qt_sb[hh * 64:(hh + 1) * 64, pp, qc * 512:(qc + 1) * 512],
                        start=True, stop=True,
                    )
                pt = ptp.tile([128, 2, 512], BF16, tag="pt")
                nc.scalar.activation(
                    pt[:, :, :], slab[:, :, :],
                    mybir.ActivationFunctionType.Exp,
                    bias=zero_t[:, :], scale=SCALE,
                )
                if pend[0] is not None:
                    flush_pv(pend[0])
                    if pop_piece:
                        # debt-based smearing: each unit has ~220ns of PE
                        # slack under the exp; pop pieces against that budget
                        pe_cap[0] = min(pe_cap[0] + 220, 2400)
                        npop = 0
                        while (finish_pieces and npop < 2
                               and finish_pieces[0][0] <= pe_cap[0]):
                            cost, fn = finish_pieces.pop(0)
                            fn()
                            pe_cap[0] -= cost
                            npop += 1
                unit_no[0] += 1
                pend[0] = (qc, pp, ci, pt)

            # ================= schedule =================
            # Phase A: tiles 0-3 (q,k,v) — ACT path (exp stream not started)
            for i in range(NQ_EARLY):
                ps, psq = tile_gemm(i, True, gemp)
                flush_ph1()
                tile_chain(i, ps, psq, "act")

            # Phase B: tiles 4-15 (k,v) interleaved with (qc0, pp0) units.
            # Order per iteration: GEMM first (its slab-release wait overlaps
            # the previous iteration's work), stage_b flush, then units (their
            # kt/qt transposes are 2+ tiles old), then the tile's post chain.
            # kv slabs alternate between the two PSUM pools so production is
            # not single-buffered.
            emitted = 0
            for i in range(NQ_EARLY, NT):
                ps, _ = tile_gemm(i, False, workp if (i % 2) else gemp)
                flush_ph1()
                while emitted < max(0, i - 2):
                    emit_unit(0, 0, emitted, pop_piece=False)
                    emitted += 1
                tile_chain(i, ps, None, "mix")
            flush_ph1()

            # queue deferred q pieces (x resident — no reloads needed)
            for qi in range(NQ_EARLY, NT):
                finish_pieces.append((1350, q_piece_a(qi)))
                finish_pieces.append((150, q_piece_b(qi)))
            # fill the kt15-transpose wait at the B->C seam with piece work
            finish_pieces.pop(0)[1]()
            finish_pieces.pop(0)[1]()

            # Phase C: remaining units
            for qc in range(NQC):
                for pp in range(3):
                    ci0 = emitted if (qc == 0 and pp == 0) else 0
                    emitted = -1
                    for ci in range(ci0, NCI):
                        emit_unit(qc, pp, ci)
            flush_pv(pend[0])
            for t in range(4):
                proj_piece(3, t, t + 1, workp if (t % 2) else gemp,
                           tail=True)()
            while finish_pieces:
                finish_pieces.pop(0)[1]()

    split_multi_waits(nc)
    return nc


# ---------------------------------------------------------------- entry
def kernel(x, qkv_w, qkv_b, proj_w, proj_b, q_norm_w, k_norm_w, _trace=False,
           _debug=False):
    x = np.asarray(x, dtype=np.float32)
    qkv_w = np.asarray(qkv_w, dtype=np.float32)
    qkv_b = np.asarray(qkv_b, dtype=np.float32)
    proj_w = np.asarray(proj_w, dtype=np.float32)
    proj_b = np.asarray(proj_b, dtype=np.float32)
    q_norm_w = np.asarray(q_norm_w, dtype=np.float32)
    k_norm_w = np.asarray(k_norm_w, dtype=np.float32)

    use_bias = bool(np.any(qkv_b != 0.0))
    key = use_bias
    if key not in _CACHE:
        _CACHE[key] = build_nc(use_bias)
    nc = _CACHE[key]
    FC = 7 if use_bias else 6

    cosq, sinq = _rope_tables(q_norm_w)
    cosk, sink = _rope_tables(k_norm_w)
    cost = np.concatenate([cosq, cosk], axis=1)  # [N, 128]
    sint = np.concatenate([sinq, sink], axis=1)

    bf16 = ml_dtypes.bfloat16
    in_maps = []
    for core in range(8):
        b, hg = core // 2, core % 2
        h0 = hg * HPC
        cols = slice(h0 * HD, (h0 + HPC) * HD)
        xt = np.ascontiguousarray(x[b].T)                       # [768, N]
        wq = qkv_w[:, cols]
        wk = qkv_w[:, D:][:, cols]
        wv = qkv_w[:, 2 * D:][:, cols]
        if use_bias:
            pad = np.zeros((128, N), np.float32)
            pad[0, :] = 1.0
            xt = np.concatenate([xt, pad], axis=0)
            wpad = np.zeros((128, HPC * HD), np.float32)
            wqb = np.concatenate([wq, wpad], axis=0)
            wkb = np.concatenate([wk, wpad], axis=0)
            wvb = np.concatenate([wv, wpad], axis=0)
            wqb[D, :] = qkv_b[cols]
            wkb[D, :] = qkv_b[D:][cols]
            wvb[D, :] = qkv_b[2 * D:][cols]
            wq, wk, wv = wqb, wkb, wvb
        wo = proj_w[h0 * HD:(h0 + HPC) * HD, :]
        im = {
            "xt": xt.astype(bf16),
            "wq": np.ascontiguousarray(wq).astype(bf16),
            "wk": np.ascontiguousarray(wk).astype(bf16),
            "wv": np.ascontiguousarray(wv).astype(bf16),
            "wo": np.ascontiguousarray(wo).astype(bf16),
            "cost": cost.astype(bf16), "sint": sint.astype(bf16),
        }
        in_maps.append(im)

    res = run_bass_kernel_spmd(nc, in_maps, core_ids=list(range(8)),
                               trace=_trace or KERNEL_TRACE)
    kernel._last = res

    y = np.empty((B, N, D), dtype=np.float32)
    for b in range(B):
        y[b] = res.results[2 * b]["y"] + res.results[2 * b + 1]["y"] + proj_b[None, :]
    return y


# revision 34
# speedup vs baseline: 1.0086x; 1.0086x over previous
"""Multi-head attention (RMSNorm-QK + RoPE) Trainium2 Bass kernel — v3.

Sharding: 8 cores = 4 batches x 2 head-groups (6 heads each).
Host sums the two partial y's per batch and adds proj bias.

v3 design (vs v2):
  - QK slabs shrink to 2 banks (2 fills/chunk, 192 units): frees 2 PSUM
    banks for a dedicated piece pool, so deferred-q GEMMs / broadcast /
    projection matmuls no longer stall the QK->exp->PV rotation.
  - QK fill pairs are (kt,hh0)+(kt,hh1): every pair runs concurrently in
    disjoint PE row groups.
  - Attention units for (qc0,pp0) are interleaved into phase-1 tile
    production, so the scalar-engine exp stream starts at ~25us.
  - Softmax denominators are gathered onto partitions 0..5 by DVE row
    copies, ln/exp runs 6-lane (was 1-lane), reciprocal rows copied back
    to partition 64 for the broadcast matmuls.
  - vaug copy and the phase-1 PSUM->SBUF q/k copy run on DVE during the
    interleaved phase to keep ACT pure-exp.
"""

import sys

for _p in ("/opt/trn_rl_repo", "/root/.axon_site/_ro/trn_rl_repo"):
    if _p not in sys.path:
        sys.path.insert(0, _p)

import numpy as np
import ml_dtypes

import bass_rust
import concourse.bass as bass
import concourse.mybir as mybir
import concourse.tile as tile
from concourse.bass_utils import run_bass_kernel_spmd

# Problem constants (hardcoded per contract)
B, N, D = 4, 2048, 768
H, HD = 12, 64
HPC = 6              # heads per core
NT = N // 128        # 16 seq tiles
EPS = 1e-6
THETA = 10000.0
SCALE = HD ** -0.5   # 0.125

F32 = mybir.dt.float32
F32R = mybir.dt.float32r
BF16 = mybir.dt.bfloat16

KERNEL_TRACE = False
_CACHE = {}


# ---------------------------------------------------------------- wait split
_ctr = [0]


def _mk_nop(engine, waits=None, updates=None):
    _ctr[0] += 1
    si = mybir.SyncInfo(on_wait=waits or [], on_update=updates or [])
    return bass_rust.InstNoOp(
        name=f"I-waitfix-{_ctr[0]}", engine=engine, ins=[], outs=[], sync_info=si
    )


def split_multi_waits(nc):
    """This walrus build accepts only ONE sync wait/update per instruction;
    hoist extras onto adjacent same-engine NoOp carriers."""
    for fn in nc.m.functions:
        for bb in fn.blocks:
            insts = bb.instructions
            out = []
            changed = False
            for inst in insts:
                si = inst.sync_info
                if si is None:
                    out.append(inst)
                    continue
                waits = list(si.on_wait or [])
                updates = list(si.on_update or [])
                pre, post = [], []
                if len(waits) > 1:
                    for w in waits[:-1]:
                        pre.append(_mk_nop(inst.engine, waits=[w]))
                    si.on_wait = [waits[-1]]
                    changed = True
                if len(updates) > 1:
                    if inst.opcode == "DMACopy":
                        raise RuntimeError(
                            f"DMACopy {inst.name} has {len(updates)} updates"
                        )
                    for u in updates[1:]:
                        post.append(_mk_nop(inst.engine, updates=[u]))
                    si.on_update = [updates[0]]
                    changed = True
                out.extend(pre)
                out.append(inst)
                out.extend(post)
            if changed:
                insts[:] = out
    return nc


# ---------------------------------------------------------------- host utils
def _rope_tables(norm_w: np.ndarray):
    """cosw[n,d] = cos[n,d]*w[d];  sinw folds the rotate-half sign+swap of w:
    q' = qn*cosw + shuffle32(qn)*sinw  (shuffle32 = swap halves, no negation)."""
    inv_freq = 1.0 / (THETA ** (np.arange(0, HD, 2, dtype=np.float32) / HD))
    t = np.arange(N, dtype=np.float32)
    freqs = np.einsum("i,j->ij", t, inv_freq).astype(np.float32)
    emb = np.concatenate([freqs, freqs], axis=-1)  # [N, HD]
    cos = np.cos(emb).astype(np.float32)
    sin = np.sin(emb).astype(np.float32)
    w = norm_w.astype(np.float32)
    cosw = cos * w[None, :]
    sinw = np.empty_like(sin)
    h = HD // 2
    sinw[:, :h] = -sin[:, :h] * w[None, h:]
    sinw[:, h:] = sin[:, h:] * w[None, :h]
    return cosw, sinw


# ---------------------------------------------------------------- bass build
def build_nc(use_bias: bool):
    FC = 7 if use_bias else 6  # feature chunks of 128 (7th = bias row)
    nc = bass.Bass()

    xt_d = nc.dram_tensor("xt", [FC * 128, N], BF16, kind="ExternalInput")
    wq_d = nc.dram_tensor("wq", [FC * 128, HPC * HD], BF16, kind="ExternalInput")
    wk_d = nc.dram_tensor("wk", [FC * 128, HPC * HD], BF16, kind="ExternalInput")
    wv_d = nc.dram_tensor("wv", [FC * 128, HPC * HD], BF16, kind="ExternalInput")
    wo_d = nc.dram_tensor("wo", [HPC * HD, D], BF16, kind="ExternalInput")
    # rope tables: [:, 0, :] = q variant, [:, 1, :] = k variant (norm_w folded)
    cos_d = nc.dram_tensor("cost", [N, 2 * HD], BF16, kind="ExternalInput")
    sin_d = nc.dram_tensor("sint", [N, 2 * HD], BF16, kind="ExternalInput")
    y_d = nc.dram_tensor("y", [N, D], F32, kind="ExternalOutput")

    NQC = 4              # query chunks of 512
    NCI = NT             # 16 chunks of (kt, hh0)+(kt, hh1) per (qc, pp)

    with tile.TileContext(nc) as tc:
        with (
            tc.tile_pool(name="const", bufs=1) as constp,
            tc.tile_pool(name="wts", bufs=1) as wts,
            tc.tile_pool(name="persist", bufs=1) as persist,
            tc.tile_pool(name="rope", bufs=2) as rope,
            tc.tile_pool(name="ptp", bufs=3) as ptp,
            tc.tile_pool(name="otunp", bufs=2) as otunp,
            tc.tile_pool(name="otqp", bufs=2) as otqp,
            tc.tile_pool(name="denp", bufs=2) as denp,
            tc.tile_pool(name="yout", bufs=2) as yout,
            tc.tile_pool(name="work", bufs=2, space="PSUM") as workp,
            tc.tile_pool(name="gemp", bufs=1, space="PSUM") as gemp,
            tc.tile_pool(name="otp", bufs=2, space="PSUM") as otp,
        ):
            # ---- constants / weights
            ones_sb = constp.tile([128, 64], BF16)
            nc.vector.memset(ones_sb[:, :], 1.0)
            eps_t = constp.tile([128, 1], F32)
            nc.vector.memset(eps_t[:, :], EPS)
            zero_t = constp.tile([128, 1], F32)
            nc.vector.memset(zero_t[:, :], 0.0)

            # x stays SBUF-resident: 4 quarter-DMAs (one trigger each; each
            # spreads over all 16 hw queues).  First GEMM needs only quarter 0
            # plus wk/wv — ~5 triggers instead of 25.
            xt_full = persist.tile([128, FC, N], BF16, tag="xtf")
            xt_dr = xt_d.rearrange("(c p) n -> p c n", p=128)

            def load_xq(q):
                nc.sync.dma_start(
                    xt_full[:, :, q * 512:(q + 1) * 512],
                    xt_dr[:, :, q * 512:(q + 1) * 512],
                )

            load_xq(0)
            w_sbs = []
            for wd, nm in ((wk_d, "wk"), (wv_d, "wv"), (wq_d, "wq")):
                wsb = wts.tile([128, FC, HPC * HD], BF16, tag=nm, name=nm)
                nc.sync.dma_start(
                    wsb[:, :, :], wd.rearrange("(c p) n -> p c n", p=128))
                w_sbs.append(wsb)
            w_sbs = [w_sbs[2], w_sbs[0], w_sbs[1]]  # back to q, k, v order
            # rope tables (bf16 host-side)
            cos_sb = constp.tile([128, NT, 2, HD], BF16, tag="cos")
            nc.sync.dma_start(
                cos_sb[:, :, :, :],
                cos_d.rearrange("(t p) (a d) -> p t a d", p=128, a=2),
            )
            sin_sb = constp.tile([128, NT, 2, HD], BF16, tag="sin")
            nc.sync.dma_start(
                sin_sb[:, :, :, :],
                sin_d.rearrange("(t p) (a d) -> p t a d", p=128, a=2),
            )
            wo_sb = wts.tile([128, 3, D], BF16, tag="wo")
            nc.sync.dma_start(wo_sb[:, :, :], wo_d.rearrange("(c p) n -> p c n", p=128))
            for _q in range(1, 4):
                load_xq(_q)

            qt_sb = persist.tile([128, 3, N], BF16, tag="qt")
            kt_sb = persist.tile([128, 3, N], BF16, tag="kt")
            vaug = persist.tile([128, NT, HPC, 65], BF16, tag="vaug")
            nc.vector.memset(vaug[:, :, :, 64:65], 1.0)

            # ================= phase 1 helpers =================
            NQ_EARLY = 4

            def ph1_stage_b(p):
                # p = (i, na, v0, ss, c2); variants v0..v0+na-1 (0=q, 1=k)
                i, na, v0, ss, c2 = p
                lg = rope.tile([128, na, HPC], F32, tag="lg")
                nc.scalar.activation(lg[:, :, :], ss[:, :, :],
                                     mybir.ActivationFunctionType.Ln,
                                     bias=eps_t[:, :], scale=1.0 / HD)
                rs = rope.tile([128, na, HPC], BF16, tag="rs")
                nc.scalar.activation(rs[:, :, :], lg[:, :, :],
                                     mybir.ActivationFunctionType.Exp,
                                     bias=zero_t[:, :], scale=-0.5)
                ro = rope.tile([128, na, HPC, HD], BF16, tag="ro")
                nc.vector.tensor_mul(
                    ro[:, :, :, :], c2[:, :, :, :],
                    rs[:, :, :, None].to_broadcast((128, na, HPC, HD)),
                )
                rof = ro.rearrange("p a h d -> p (a h d)")
                for j in range(na):
                    dst = qt_sb if v0 + j == 0 else kt_sb
                    nc.sync.dma_start_transpose(
                        dst[:, 0:3, i * 128:(i + 1) * 128],
                        rof[:, j * 384:(j + 1) * 384])

            def norm_rope(i, ps_ap, na, v0, path):
                """ps_ap: PSUM [128, na, 384] raw q/k; returns (ss, c2).
                path: 'act' = copy+square on ACT, 'dve' = both on DVE,
                'mix' = copy on DVE + square on ACT."""
                src = ps_ap.rearrange("p a (h d) -> p a h d", h=HPC)
                qk = rope.tile([128, na, HPC, HD], BF16, tag="qk")
                sq = rope.tile([128, na, HPC, HD], BF16, tag="sq")
                if path == "act":
                    nc.scalar.copy(qk[:, :, :, :], src)
                    nc.scalar.activation(sq[:, :, :, :], src,
                                         mybir.ActivationFunctionType.Square,
                                         bias=zero_t[:, :])
                elif path == "mix":
                    nc.vector.tensor_copy(qk[:, :, :, :], src)
                    nc.scalar.activation(sq[:, :, :, :], src,
                                         mybir.ActivationFunctionType.Square,
                                         bias=zero_t[:, :])
                else:
                    nc.vector.tensor_copy(qk[:, :, :, :], src)
                    nc.vector.tensor_mul(sq[:, :, :, :], qk[:, :, :, :],
                                         qk[:, :, :, :])
                ss = rope.tile([128, na, HPC], F32, tag="ss")
                nc.vector.reduce_sum(ss[:, :, :], sq[:, :, :, :],
                                     axis=mybir.AxisListType.X)
                cosb = cos_sb[:, i, v0:v0 + na, None, :].to_broadcast(
                    (128, na, HPC, HD))
                sinb = sin_sb[:, i, v0:v0 + na, None, :]
                a = rope.tile([128, na, HPC, HD], BF16, tag="a")
                nc.vector.tensor_mul(a[:, :, :, :], qk[:, :, :, :], cosb)
                bt = rope.tile([128, na, HPC, HD], BF16, tag="bt")
                h = HD // 2
                nc.vector.tensor_mul(
                    bt[:, :, :, 0:h], qk[:, :, :, h:HD],
                    sinb[:, :, :, 0:h].to_broadcast((128, na, HPC, h)))
                nc.vector.tensor_mul(
                    bt[:, :, :, h:HD], qk[:, :, :, 0:h],
                    sinb[:, :, :, h:HD].to_broadcast((128, na, HPC, h)))
                c2 = rope.tile([128, na, HPC, HD], BF16, tag="c2")
                nc.vector.tensor_add(c2[:, :, :, :], a[:, :, :, :],
                                     bt[:, :, :, :])
                return ss, c2

            ph1_pend = []

            def flush_ph1():
                for pd in ph1_pend:
                    ph1_stage_b(pd)
                ph1_pend.clear()

            def tile_gemm(i, with_q, kv_pool):
                """K+V GEMM (+Q first, for early tiles) for seq tile i."""
                xs = xt_full[:, :, i * 128:(i + 1) * 128]
                psq = None
                if with_q:
                    # Q leads: qc0 units can't start until all four early q
                    # transposes land, so q's chain goes first everywhere.
                    psq = workp.tile([128, 2, 512], F32, tag="work",
                                     name=f"q{i}")
                    for c in range(FC):
                        nc.tensor.matmul(
                            psq[:, 0, 0:384], xs[:, c, :], w_sbs[0][:, c, :],
                            start=(c == 0), stop=(c == FC - 1),
                        )
                ps = kv_pool.tile([128, 2, 512], F32,
                                  tag="gp" if kv_pool is gemp else "work",
                                  name=f"kv{i}")
                for c in range(FC):
                    for t, wsb in enumerate((w_sbs[1], w_sbs[2])):
                        nc.tensor.matmul(
                            ps[:, t, 0:384], xs[:, c, :], wsb[:, c, :],
                            start=(c == 0), stop=(c == FC - 1),
                        )
                return ps, psq

            def tile_chain(i, ps, psq, path):
                pend_local = []
                if psq is not None:
                    ssq, c2q = norm_rope(i, psq[:, 0:1, 0:384], 1, 0, path)
                    pend_local.append((i, 1, 0, ssq, c2q))
                # V copy to vaug (DVE keeps ACT free for exp)
                nc.vector.tensor_copy(
                    vaug[:, i, :, 0:64],
                    ps[:, 1, 0:384].rearrange("p (h d) -> p h d", h=HPC),
                )
                ssk, c2k = norm_rope(i, ps[:, 0:1, 0:384], 1, 1, path)
                pend_local.append((i, 1, 1, ssk, c2k))
                ph1_pend.extend(pend_local)

            # ================= attention unit machinery =================
            cur_ots = {}
            otun_by_qc = {}
            den_by_qc = {}       # qc -> (den6 [6,512] f32, rec6 [6,512] bf16)
            rec_by_qc = {}       # qc -> rec64 [65, HPC, 512] bf16
            otq_by_qc = {}
            finish_pieces = []
            pend = [None]        # (qc, pp, ci, pt)

            def emit_den6(qc):
                # Gather the 6 den rows (partition 64 of otun) onto partitions
                # 0..5 via tiny SBUF->SBUF DMAs (engines can't address base
                # partitions outside {0,32,64,96}; DMA can), ln/exp 6-lane,
                # then scatter the reciprocal rows back to partition 64.
                otun_all = otun_by_qc[qc]
                den6i = denp.tile([6, 512], BF16, tag="den6i",
                                  name=f"den6i_{qc}")
                lnt = denp.tile([6, 512], F32, tag="lnt", name=f"lnt_{qc}")
                rec6 = denp.tile([6, 512], BF16, tag="rec6", name=f"rec6_{qc}")
                rec64 = rec_by_qc.setdefault(
                    qc, denp.tile([65, HPC, 512], BF16, tag="rec64",
                                  name=f"rec64_{qc}"))
                for hloc in range(HPC):
                    nc.sync.dma_start(den6i[hloc:hloc + 1, :],
                                      otun_all[64:65, hloc, :])
                nc.scalar.activation(lnt[0:6, :], den6i[0:6, :],
                                     mybir.ActivationFunctionType.Ln,
                                     bias=zero_t[0:6, :], scale=1.0)
                nc.scalar.activation(rec6[0:6, :], lnt[0:6, :],
                                     mybir.ActivationFunctionType.Exp,
                                     bias=zero_t[0:6, :], scale=-1.0)
                for hloc in range(HPC):
                    nc.sync.dma_start(rec64[64:65, hloc, :],
                                      rec6[hloc:hloc + 1, :])

            def emit_den_tail(qc, h0, h1):
                # Tail path (last qc): 1-lane ln/exp directly on the otun den
                # rows at partition 64 — no DMA latency in the critical tail.
                otun_all = otun_by_qc[qc]
                lg3 = denp.tile([65, 2, 512], F32, tag="lg3", name="lg3")
                rec64 = rec_by_qc.setdefault(
                    qc, denp.tile([65, HPC, 512], BF16, tag="rec64",
                                  name=f"rec64_{qc}"))
                nc.scalar.activation(lg3[64:65, :, :],
                                     otun_all[64:65, h0:h1, :],
                                     mybir.ActivationFunctionType.Ln,
                                     bias=zero_t[64:65, :], scale=1.0)
                nc.scalar.activation(rec64[64:65, h0:h1, :], lg3[64:65, :, :],
                                     mybir.ActivationFunctionType.Exp,
                                     bias=zero_t[64:65, :], scale=-1.0)

            def norm_piece(qc, h0, h1):
                def fn():
                    otun_all = otun_by_qc[qc]
                    rec64 = rec_by_qc[qc]
                    if qc not in otq_by_qc:
                        otq_by_qc[qc] = otqp.tile([128, 3, 512], BF16,
                                                  tag="otq", name=f"otq{qc}")
                    otq = otq_by_qc[qc]
                    bcw = gemp.tile([128, 2, 512], F32, tag="gp",
                                    name="bcw")
                    for j, hloc in enumerate(range(h0, h1)):
                        pp_, hh_ = hloc // 2, hloc % 2
                        nc.tensor.matmul(bcw[0:64, j, :],
                                         ones_sb[64:65, :],
                                         rec64[64:65, hloc, :],
                                         start=True, stop=True)
                        nc.vector.tensor_mul(
                            otq[hh_ * 64:(hh_ + 1) * 64, pp_, :],
                            otun_all[0:64, hloc, :],
                            bcw[0:64, j, :],
                        )
                return fn

            def proj_piece(qc, t0, t1, pool=None, tail=False):
                def fn():
                    pl = pool if pool is not None else gemp
                    otq = otq_by_qc[qc]
                    for qt4 in range(t0, t1):
                        q0 = qc * 512 + qt4 * 128
                        yps = pl.tile([128, 2, 512], F32,
                                      tag="gp" if pl is gemp else "work",
                                      name="yps")
                        for c in range(3):
                            nc.tensor.matmul(
                                yps[:, 0, :],
                                otq[:, c, qt4 * 128:(qt4 + 1) * 128],
                                wo_sb[:, c, 0:512],
                                start=(c == 0), stop=(c == 2),
                            )
                        for c in range(3):
                            nc.tensor.matmul(
                                yps[:, 1, 0:256],
                                otq[:, c, qt4 * 128:(qt4 + 1) * 128],
                                wo_sb[:, c, 512:768],
                                start=(c == 0), stop=(c == 2),
                            )
                        ysb = yout.tile([128, D], F32, tag="ysb")
                        if tail:
                            # split across engines — ACT is idle in the tail
                            nc.scalar.copy(ysb[:, 0:512], yps[:, 0, :])
                        else:
                            nc.vector.tensor_copy(ysb[:, 0:512], yps[:, 0, :])
                        nc.vector.tensor_copy(ysb[:, 512:768], yps[:, 1, 0:256])
                        nc.sync.dma_start(y_d[q0:q0 + 128, :], ysb[:, :])
                return fn

            # deferred Q work for tiles 4-15, as pieces (x is SBUF-resident)
            q_state = {}

            def q_piece_a(i):
                def fn():
                    psq = gemp.tile([128, 2, 512], F32, tag="gp",
                                    name=f"psq{i}")
                    for c in range(FC):
                        nc.tensor.matmul(
                            psq[:, 0, 0:384], xt_full[:, c, i * 128:(i + 1) * 128],
                            w_sbs[0][:, c, :],
                            start=(c == 0), stop=(c == FC - 1),
                        )
                    # DVE-path norm stats (ACT is saturated by exp)
                    q_state[i] = norm_rope(i, psq[:, 0:1, 0:384], 1, 0, "dve")
                return fn

            def q_piece_b(i):
                def fn():
                    ss, c2 = q_state.pop(i)
                    ph1_stage_b((i, 1, 0, ss, c2))
                return fn

            def flush_pv(p):
                qc, pp, ci, pt = p
                key = (qc, pp)
                if key not in cur_ots:
                    cur_ots[key] = [
                        otp.tile([128, 512], F32, tag="ot", name=f"ots{hh}")
                        for hh in range(2)
                    ]
                ots = cur_ots[key]
                for hh in range(2):
                    nc.tensor.matmul(
                        ots[hh][0:65, :],
                        vaug[:, ci, pp * 2 + hh, :],
                        pt[:, hh, :],
                        start=(ci == 0), stop=(ci == NCI - 1),
                    )
                if ci == NCI - 1:
                    if qc not in otun_by_qc:
                        otun_by_qc[qc] = otunp.tile(
                            [65, HPC, 512], BF16, tag="otun", name=f"otun{qc}")
                    otun_all = otun_by_qc[qc]
                    for hh in range(2):
                        nc.vector.tensor_copy(
                            otun_all[0:65, pp * 2 + hh, :], ots[hh][0:65, :])
                    del cur_ots[key]
                    if qc == 3:
                        # Last qc: den + normalize per pp so the final
                        # projection isn't one serial tail.
                        emit_den_tail(3, pp * 2, pp * 2 + 2)
                        norm_piece(3, pp * 2, pp * 2 + 2)()
                    elif pp == 2:
                        emit_den6(qc)
                elif ci == 4 and pp == 0 and qc > 0:
                    # qc-1's normalize/projection pieces: enqueue a few units
                    # into this qc (den6 DMA chain has landed by now) so they
                    # spread over ~44 piece slots instead of ~28.
                    finish_pieces.append(norm_piece(qc - 1, 0, 2))
                    finish_pieces.append(norm_piece(qc - 1, 2, 4))
                    finish_pieces.append(norm_piece(qc - 1, 4, 6))
                    for t in range(4):
                        finish_pieces.append(proj_piece(qc - 1, t, t + 1))

            unit_no = [0]

            def emit_unit(qc, pp, ci, pop_piece=True):
                slab = workp.tile([128, 2, 512], F32, tag="work", name="slab")
                for hh in range(2):
                    nc.tensor.matmul(
                        slab[:, hh, :],
                        kt_sb[hh * 64:(hh + 1) * 64, pp, ci * 128:(ci + 1) * 128],
                        qt_sb[hh * 64:(hh + 1) * 64, pp, qc * 512:(qc + 1) * 512],
                        start=True, stop=True,
                    )
                pt = ptp.tile([128, 2, 512], BF16, tag="pt")
                nc.scalar.activation(
                    pt[:, :, :], slab[:, :, :],
                    mybir.ActivationFunctionType.Exp,
                    bias=zero_t[:, :], scale=SCALE,
                )
                if pend[0] is not None:
                    flush_pv(pend[0])
                    if pop_piece and finish_pieces and unit_no[0] % 2 == 0:
                        finish_pieces.pop(0)()
                unit_no[0] += 1
                pend[0] = (qc, pp, ci, pt)

            # ================= schedule =================
            # Phase A: tiles 0-3 (q,k,v) — ACT path (exp stream not started)
            for i in range(NQ_EARLY):
                ps, psq = tile_gemm(i, True, gemp)
                flush_ph1()
                tile_chain(i, ps, psq, "act")

            # Phase B: tiles 4-15 (k,v) interleaved with (qc0, pp0) units.
            # Order per iteration: GEMM first (its slab-release wait overlaps
            # the previous iteration's work), stage_b flush, then units (their
            # kt/qt transposes are 2+ tiles old), then the tile's post chain.
            # kv slabs alternate between the two PSUM pools so production is
            # not single-buffered.
            emitted = 0
            for i in range(NQ_EARLY, NT):
                ps, _ = tile_gemm(i, False, workp if (i % 2) else gemp)
                flush_ph1()
                while emitted < max(0, i - 2):
                    emit_unit(0, 0, emitted, pop_piece=False)
                    emitted += 1
                tile_chain(i, ps, None, "mix")
            flush_ph1()

            # queue deferred q pieces (x resident — no reloads needed)
            for qi in range(NQ_EARLY, NT):
                finish_pieces.append(q_piece_a(qi))
                finish_pieces.append(q_piece_b(qi))

            # Phase C: remaining units
            for qc in range(NQC):
                for pp in range(3):
                    ci0 = emitted if (qc == 0 and pp == 0) else 0
                    emitted = -1
                    for ci in range(ci0, NCI):
                        emit_unit(qc, pp, ci)
            flush_pv(pend[0])
            for t in range(4):
                proj_piece(3, t, t + 1, workp if (t % 2) else gemp,
                           tail=True)()
            while finish_pieces:
                finish_pieces.pop(0)()

    split_multi_waits(nc)
    return nc


# ---------------------------------------------------------------- entry
def kernel(x, qkv_w, qkv_b, proj_w, proj_b, q_norm_w, k_norm_w, _trace=False,
           _debug=False):
    x = np.asarray(x, dtype=np.float32)
    qkv_w = np.asarray(qkv_w, dtype=np.float32)
    qkv_b = np.asarray(qkv_b, dtype=np.float32)
    proj_w = np.asarray(proj_w, dtype=np.float32)
    proj_b = np.asarray(proj_b, dtype=np.float32)
    q_norm_w = np.asarray(q_norm_w, dtype=np.float32)
    k_norm_w = np.asarray(k_norm_w, dtype=np.float32)

    use_bias = bool(np.any(qkv_b != 0.0))
    key = use_bias
    if key not in _CACHE:
        _CACHE[key] = build_nc(use_bias)
    nc = _CACHE[key]
    FC = 7 if use_bias else 6

    cosq, sinq = _rope_tables(q_norm_w)
    cosk, sink = _rope_tables(k_norm_w)
    cost = np.concatenate([cosq, cosk], axis=1)  # [N, 128]
    sint = np.concatenate([sinq, sink], axis=1)

    bf16 = ml_dtypes.bfloat16
    in_maps = []
    for core in range(8):
        b, hg = core // 2, core % 2
        h0 = hg * HPC
        cols = slice(h0 * HD, (h0 + HPC) * HD)
        xt = np.ascontiguousarray(x[b].T)                       # [768, N]
        wq = qkv_w[:, cols]
        wk = qkv_w[:, D:][:, cols]
        wv = qkv_w[:, 2 * D:][:, cols]
        if use_bias:
            pad = np.zeros((128, N), np.float32)
            pad[0, :] = 1.0
            xt = np.concatenate([xt, pad], axis=0)
            wpad = np.zeros((128, HPC * HD), np.float32)
            wqb = np.concatenate([wq, wpad], axis=0)
            wkb = np.concatenate([wk, wpad], axis=0)
            wvb = np.concatenate([wv, wpad], axis=0)
            wqb[D, :] = qkv_b[cols]
            wkb[D, :] = qkv_b[D:][cols]
            wvb[D, :] = qkv_b[2 * D:][cols]
            wq, wk, wv = wqb, wkb, wvb
        wo = proj_w[h0 * HD:(h0 + HPC) * HD, :]
        im = {
            "xt": xt.astype(bf16),
            "wq": np.ascontiguousarray(wq).astype(bf16),
            "wk": np.ascontiguousarray(wk).astype(bf16),
            "wv": np.ascontiguousarray(wv).astype(bf16),
            "wo": np.ascontiguousarray(wo).astype(bf16),
            "cost": cost.astype(bf16), "sint": sint.astype(bf16),
        }
        in_maps.append(im)

    res = run_bass_kernel_spmd(nc, in_maps, core_ids=list(range(8)),
                               trace=_trace or KERNEL_TRACE)
    kernel._last = res

    y = np.empty((B, N, D), dtype=np.float32)
    for b in range(B):
        y[b] = res.results[2 * b]["y"] + res.results[2 * b + 1]["y"] + proj_b[None, :]
    return y


# revision 35
# speedup vs baseline: 1.0181x; 1.0095x over previous
"""Multi-head attention (RMSNorm-QK + RoPE) Trainium2 Bass kernel — v3.

Sharding: 8 cores = 4 batches x 2 head-groups (6 heads each).
Host sums the two partial y's per batch and adds proj bias.

v3 design (vs v2):
  - QK slabs shrink to 2 banks (2 fills/chunk, 192 units): frees 2 PSUM
    banks for a dedicated piece pool, so deferred-q GEMMs / broadcast /
    projection matmuls no longer stall the QK->exp->PV rotation.
  - QK fill pairs are (kt,hh0)+(kt,hh1): every pair runs concurrently in
    disjoint PE row groups.
  - Attention units for (qc0,pp0) are interleaved into phase-1 tile
    production, so the scalar-engine exp stream starts at ~25us.
  - Softmax denominators are gathered onto partitions 0..5 by DVE row
    copies, ln/exp runs 6-lane (was 1-lane), reciprocal rows copied back
    to partition 64 for the broadcast matmuls.
  - vaug copy and the phase-1 PSUM->SBUF q/k copy run on DVE during the
    interleaved phase to keep ACT pure-exp.
"""

import sys

for _p in ("/opt/trn_rl_repo", "/root/.axon_site/_ro/trn_rl_repo"):
    if _p not in sys.path:
        sys.path.insert(0, _p)

import numpy as np
import ml_dtypes

import bass_rust
import concourse.bass as bass
import concourse.mybir as mybir
import concourse.tile as tile
from concourse.bass_utils import run_bass_kernel_spmd

# Problem constants (hardcoded per contract)
B, N, D = 4, 2048, 768
H, HD = 12, 64
HPC = 6              # heads per core
NT = N // 128        # 16 seq tiles
EPS = 1e-6
THETA = 10000.0
SCALE = HD ** -0.5   # 0.125

F32 = mybir.dt.float32
F32R = mybir.dt.float32r
BF16 = mybir.dt.bfloat16

KERNEL_TRACE = False
_CACHE = {}


# ---------------------------------------------------------------- wait split
_ctr = [0]


def _mk_nop(engine, waits=None, updates=None):
    _ctr[0] += 1
    si = mybir.SyncInfo(on_wait=waits or [], on_update=updates or [])
    return bass_rust.InstNoOp(
        name=f"I-waitfix-{_ctr[0]}", engine=engine, ins=[], outs=[], sync_info=si
    )


def split_multi_waits(nc):
    """This walrus build accepts only ONE sync wait/update per instruction;
    hoist extras onto adjacent same-engine NoOp carriers."""
    for fn in nc.m.functions:
        for bb in fn.blocks:
            insts = bb.instructions
            out = []
            changed = False
            for inst in insts:
                si = inst.sync_info
                if si is None:
                    out.append(inst)
                    continue
                waits = list(si.on_wait or [])
                updates = list(si.on_update or [])
                pre, post = [], []
                if len(waits) > 1:
                    for w in waits[:-1]:
                        pre.append(_mk_nop(inst.engine, waits=[w]))
                    si.on_wait = [waits[-1]]
                    changed = True
                if len(updates) > 1:
                    if inst.opcode == "DMACopy":
                        raise RuntimeError(
                            f"DMACopy {inst.name} has {len(updates)} updates"
                        )
                    for u in updates[1:]:
                        post.append(_mk_nop(inst.engine, updates=[u]))
                    si.on_update = [updates[0]]
                    changed = True
                out.extend(pre)
                out.append(inst)
                out.extend(post)
            if changed:
                insts[:] = out
    return nc


# ---------------------------------------------------------------- host utils
def _rope_tables(norm_w: np.ndarray):
    """cosw[n,d] = cos[n,d]*w[d];  sinw folds the rotate-half sign+swap of w:
    q' = qn*cosw + shuffle32(qn)*sinw  (shuffle32 = swap halves, no negation)."""
    inv_freq = 1.0 / (THETA ** (np.arange(0, HD, 2, dtype=np.float32) / HD))
    t = np.arange(N, dtype=np.float32)
    freqs = np.einsum("i,j->ij", t, inv_freq).astype(np.float32)
    emb = np.concatenate([freqs, freqs], axis=-1)  # [N, HD]
    cos = np.cos(emb).astype(np.float32)
    sin = np.sin(emb).astype(np.float32)
    w = norm_w.astype(np.float32)
    cosw = cos * w[None, :]
    sinw = np.empty_like(sin)
    h = HD // 2
    sinw[:, :h] = -sin[:, :h] * w[None, h:]
    sinw[:, h:] = sin[:, h:] * w[None, :h]
    return cosw, sinw


# ---------------------------------------------------------------- bass build
def build_nc(use_bias: bool):
    FC = 7 if use_bias else 6  # feature chunks of 128 (7th = bias row)
    nc = bass.Bass()

    xt_d = nc.dram_tensor("xt", [FC * 128, N], BF16, kind="ExternalInput")
    wq_d = nc.dram_tensor("wq", [FC * 128, HPC * HD], BF16, kind="ExternalInput")
    wk_d = nc.dram_tensor("wk", [FC * 128, HPC * HD], BF16, kind="ExternalInput")
    wv_d = nc.dram_tensor("wv", [FC * 128, HPC * HD], BF16, kind="ExternalInput")
    wo_d = nc.dram_tensor("wo", [HPC * HD, D], BF16, kind="ExternalInput")
    # rope tables: [:, 0, :] = q variant, [:, 1, :] = k variant (norm_w folded)
    cos_d = nc.dram_tensor("cost", [N, 2 * HD], BF16, kind="ExternalInput")
    sin_d = nc.dram_tensor("sint", [N, 2 * HD], BF16, kind="ExternalInput")
    y_d = nc.dram_tensor("y", [N, D], F32, kind="ExternalOutput")

    NQC = 4              # query chunks of 512
    NCI = NT             # 16 chunks of (kt, hh0)+(kt, hh1) per (qc, pp)

    with tile.TileContext(nc) as tc:
        with (
            tc.tile_pool(name="const", bufs=1) as constp,
            tc.tile_pool(name="wts", bufs=1) as wts,
            tc.tile_pool(name="persist", bufs=1) as persist,
            tc.tile_pool(name="rope", bufs=3) as rope,
            tc.tile_pool(name="ptp", bufs=4) as ptp,
            tc.tile_pool(name="otunp", bufs=2) as otunp,
            tc.tile_pool(name="otqp", bufs=2) as otqp,
            tc.tile_pool(name="denp", bufs=2) as denp,
            tc.tile_pool(name="yout", bufs=2) as yout,
            tc.tile_pool(name="work", bufs=2, space="PSUM") as workp,
            tc.tile_pool(name="gemp", bufs=1, space="PSUM") as gemp,
            tc.tile_pool(name="otp", bufs=2, space="PSUM") as otp,
        ):
            # ---- constants / weights
            ones_sb = constp.tile([128, 64], BF16)
            nc.vector.memset(ones_sb[:, :], 1.0)
            eps_t = constp.tile([128, 1], F32)
            nc.vector.memset(eps_t[:, :], EPS)
            zero_t = constp.tile([128, 1], F32)
            nc.vector.memset(zero_t[:, :], 0.0)

            # x stays SBUF-resident: 4 quarter-DMAs (one trigger each; each
            # spreads over all 16 hw queues).  First GEMM needs only quarter 0
            # plus wk/wv — ~5 triggers instead of 25.
            xt_full = persist.tile([128, FC, N], BF16, tag="xtf")
            xt_dr = xt_d.rearrange("(c p) n -> p c n", p=128)

            def load_xq(q):
                nc.sync.dma_start(
                    xt_full[:, :, q * 512:(q + 1) * 512],
                    xt_dr[:, :, q * 512:(q + 1) * 512],
                )

            load_xq(0)
            w_sbs = []
            for wd, nm in ((wk_d, "wk"), (wv_d, "wv"), (wq_d, "wq")):
                wsb = wts.tile([128, FC, HPC * HD], BF16, tag=nm, name=nm)
                nc.sync.dma_start(
                    wsb[:, :, :], wd.rearrange("(c p) n -> p c n", p=128))
                w_sbs.append(wsb)
            w_sbs = [w_sbs[2], w_sbs[0], w_sbs[1]]  # back to q, k, v order
            # rope tables (bf16 host-side)
            cos_sb = constp.tile([128, NT, 2, HD], BF16, tag="cos")
            nc.sync.dma_start(
                cos_sb[:, :, :, :],
                cos_d.rearrange("(t p) (a d) -> p t a d", p=128, a=2),
            )
            sin_sb = constp.tile([128, NT, 2, HD], BF16, tag="sin")
            nc.sync.dma_start(
                sin_sb[:, :, :, :],
                sin_d.rearrange("(t p) (a d) -> p t a d", p=128, a=2),
            )
            wo_sb = wts.tile([128, 3, D], BF16, tag="wo")
            nc.sync.dma_start(wo_sb[:, :, :], wo_d.rearrange("(c p) n -> p c n", p=128))
            for _q in range(1, 4):
                load_xq(_q)

            qt_sb = persist.tile([128, 3, N], BF16, tag="qt")
            kt_sb = persist.tile([128, 3, N], BF16, tag="kt")
            vaug = persist.tile([128, NT, HPC, 65], BF16, tag="vaug")
            nc.vector.memset(vaug[:, :, :, 64:65], 1.0)

            # ================= phase 1 helpers =================
            NQ_EARLY = 4

            def ph1_stage_b(p):
                # p = (i, na, v0, ss, c2); variants v0..v0+na-1 (0=q, 1=k)
                i, na, v0, ss, c2 = p
                lg = rope.tile([128, na, HPC], F32, tag="lg")
                nc.scalar.activation(lg[:, :, :], ss[:, :, :],
                                     mybir.ActivationFunctionType.Ln,
                                     bias=eps_t[:, :], scale=1.0 / HD)
                rs = rope.tile([128, na, HPC], BF16, tag="rs")
                nc.scalar.activation(rs[:, :, :], lg[:, :, :],
                                     mybir.ActivationFunctionType.Exp,
                                     bias=zero_t[:, :], scale=-0.5)
                ro = rope.tile([128, na, HPC, HD], BF16, tag="ro")
                nc.vector.tensor_mul(
                    ro[:, :, :, :], c2[:, :, :, :],
                    rs[:, :, :, None].to_broadcast((128, na, HPC, HD)),
                )
                rof = ro.rearrange("p a h d -> p (a h d)")
                for j in range(na):
                    dst = qt_sb if v0 + j == 0 else kt_sb
                    nc.sync.dma_start_transpose(
                        dst[:, 0:3, i * 128:(i + 1) * 128],
                        rof[:, j * 384:(j + 1) * 384])

            def norm_rope(i, ps_ap, na, v0, path):
                """ps_ap: PSUM [128, na, 384] raw q/k; returns (ss, c2).
                path: 'act' = copy+square on ACT, 'dve' = both on DVE,
                'mix' = copy on DVE + square on ACT."""
                src = ps_ap.rearrange("p a (h d) -> p a h d", h=HPC)
                qk = rope.tile([128, na, HPC, HD], BF16, tag="qk")
                sq = rope.tile([128, na, HPC, HD], BF16, tag="sq")
                if path == "act":
                    nc.scalar.copy(qk[:, :, :, :], src)
                    nc.scalar.activation(sq[:, :, :, :], src,
                                         mybir.ActivationFunctionType.Square,
                                         bias=zero_t[:, :])
                elif path == "mix":
                    nc.vector.tensor_copy(qk[:, :, :, :], src)
                    nc.scalar.activation(sq[:, :, :, :], src,
                                         mybir.ActivationFunctionType.Square,
                                         bias=zero_t[:, :])
                else:
                    nc.vector.tensor_copy(qk[:, :, :, :], src)
                    nc.vector.tensor_mul(sq[:, :, :, :], qk[:, :, :, :],
                                         qk[:, :, :, :])
                ss = rope.tile([128, na, HPC], F32, tag="ss")
                nc.vector.reduce_sum(ss[:, :, :], sq[:, :, :, :],
                                     axis=mybir.AxisListType.X)
                cosb = cos_sb[:, i, v0:v0 + na, None, :].to_broadcast(
                    (128, na, HPC, HD))
                sinb = sin_sb[:, i, v0:v0 + na, None, :]
                a = rope.tile([128, na, HPC, HD], BF16, tag="a")
                nc.vector.tensor_mul(a[:, :, :, :], qk[:, :, :, :], cosb)
                bt = rope.tile([128, na, HPC, HD], BF16, tag="bt")
                h = HD // 2
                nc.vector.tensor_mul(
                    bt[:, :, :, 0:h], qk[:, :, :, h:HD],
                    sinb[:, :, :, 0:h].to_broadcast((128, na, HPC, h)))
                nc.vector.tensor_mul(
                    bt[:, :, :, h:HD], qk[:, :, :, 0:h],
                    sinb[:, :, :, h:HD].to_broadcast((128, na, HPC, h)))
                c2 = rope.tile([128, na, HPC, HD], BF16, tag="c2")
                nc.vector.tensor_add(c2[:, :, :, :], a[:, :, :, :],
                                     bt[:, :, :, :])
                return ss, c2

            ph1_pend = []

            def flush_ph1():
                for pd in ph1_pend:
                    ph1_stage_b(pd)
                ph1_pend.clear()

            def tile_gemm(i, with_q, kv_pool):
                """K+V GEMM (+Q first, for early tiles) for seq tile i."""
                xs = xt_full[:, :, i * 128:(i + 1) * 128]
                psq = None
                if with_q:
                    # Q leads: qc0 units can't start until all four early q
                    # transposes land, so q's chain goes first everywhere.
                    psq = workp.tile([128, 2, 512], F32, tag="work",
                                     name=f"q{i}")
                    for c in range(FC):
                        nc.tensor.matmul(
                            psq[:, 0, 0:384], xs[:, c, :], w_sbs[0][:, c, :],
                            start=(c == 0), stop=(c == FC - 1),
                        )
                ps = kv_pool.tile([128, 2, 512], F32,
                                  tag="gp" if kv_pool is gemp else "work",
                                  name=f"kv{i}")
                for c in range(FC):
                    for t, wsb in enumerate((w_sbs[1], w_sbs[2])):
                        nc.tensor.matmul(
                            ps[:, t, 0:384], xs[:, c, :], wsb[:, c, :],
                            start=(c == 0), stop=(c == FC - 1),
                        )
                return ps, psq

            def tile_chain(i, ps, psq, path):
                pend_local = []
                if psq is not None:
                    ssq, c2q = norm_rope(i, psq[:, 0:1, 0:384], 1, 0, path)
                    pend_local.append((i, 1, 0, ssq, c2q))
                # V copy to vaug (DVE keeps ACT free for exp)
                nc.vector.tensor_copy(
                    vaug[:, i, :, 0:64],
                    ps[:, 1, 0:384].rearrange("p (h d) -> p h d", h=HPC),
                )
                ssk, c2k = norm_rope(i, ps[:, 0:1, 0:384], 1, 1, path)
                pend_local.append((i, 1, 1, ssk, c2k))
                ph1_pend.extend(pend_local)

            # ================= attention unit machinery =================
            cur_ots = {}
            otun_by_qc = {}
            den_by_qc = {}       # qc -> (den6 [6,512] f32, rec6 [6,512] bf16)
            rec_by_qc = {}       # qc -> rec64 [65, HPC, 512] bf16
            otq_by_qc = {}
            finish_pieces = []
            pend = [None]        # (qc, pp, ci, pt)

            def emit_den6(qc):
                # Gather the 6 den rows (partition 64 of otun) onto partitions
                # 0..5 via tiny SBUF->SBUF DMAs (engines can't address base
                # partitions outside {0,32,64,96}; DMA can), ln/exp 6-lane,
                # then scatter the reciprocal rows back to partition 64.
                otun_all = otun_by_qc[qc]
                den6i = denp.tile([6, 512], BF16, tag="den6i",
                                  name=f"den6i_{qc}")
                lnt = denp.tile([6, 512], F32, tag="lnt", name=f"lnt_{qc}")
                rec6 = denp.tile([6, 512], BF16, tag="rec6", name=f"rec6_{qc}")
                rec64 = rec_by_qc.setdefault(
                    qc, denp.tile([65, HPC, 512], BF16, tag="rec64",
                                  name=f"rec64_{qc}"))
                for hloc in range(HPC):
                    nc.sync.dma_start(den6i[hloc:hloc + 1, :],
                                      otun_all[64:65, hloc, :])
                nc.scalar.activation(lnt[0:6, :], den6i[0:6, :],
                                     mybir.ActivationFunctionType.Ln,
                                     bias=zero_t[0:6, :], scale=1.0)
                nc.scalar.activation(rec6[0:6, :], lnt[0:6, :],
                                     mybir.ActivationFunctionType.Exp,
                                     bias=zero_t[0:6, :], scale=-1.0)
                for hloc in range(HPC):
                    nc.sync.dma_start(rec64[64:65, hloc, :],
                                      rec6[hloc:hloc + 1, :])

            def emit_den_tail(qc, h0, h1):
                # Tail path (last qc): 1-lane ln/exp directly on the otun den
                # rows at partition 64 — no DMA latency in the critical tail.
                otun_all = otun_by_qc[qc]
                lg3 = denp.tile([65, 2, 512], F32, tag="lg3", name="lg3")
                rec64 = rec_by_qc.setdefault(
                    qc, denp.tile([65, HPC, 512], BF16, tag="rec64",
                                  name=f"rec64_{qc}"))
                nc.scalar.activation(lg3[64:65, :, :],
                                     otun_all[64:65, h0:h1, :],
                                     mybir.ActivationFunctionType.Ln,
                                     bias=zero_t[64:65, :], scale=1.0)
                nc.scalar.activation(rec64[64:65, h0:h1, :], lg3[64:65, :, :],
                                     mybir.ActivationFunctionType.Exp,
                                     bias=zero_t[64:65, :], scale=-1.0)

            def norm_piece(qc, h0, h1):
                def fn():
                    otun_all = otun_by_qc[qc]
                    rec64 = rec_by_qc[qc]
                    if qc not in otq_by_qc:
                        otq_by_qc[qc] = otqp.tile([128, 3, 512], BF16,
                                                  tag="otq", name=f"otq{qc}")
                    otq = otq_by_qc[qc]
                    bcw = gemp.tile([128, 2, 512], F32, tag="gp",
                                    name="bcw")
                    for j, hloc in enumerate(range(h0, h1)):
                        pp_, hh_ = hloc // 2, hloc % 2
                        nc.tensor.matmul(bcw[0:64, j, :],
                                         ones_sb[64:65, :],
                                         rec64[64:65, hloc, :],
                                         start=True, stop=True)
                        nc.vector.tensor_mul(
                            otq[hh_ * 64:(hh_ + 1) * 64, pp_, :],
                            otun_all[0:64, hloc, :],
                            bcw[0:64, j, :],
                        )
                return fn

            def proj_piece(qc, t0, t1, pool=None, tail=False):
                def fn():
                    pl = pool if pool is not None else gemp
                    otq = otq_by_qc[qc]
                    for qt4 in range(t0, t1):
                        q0 = qc * 512 + qt4 * 128
                        yps = pl.tile([128, 2, 512], F32,
                                      tag="gp" if pl is gemp else "work",
                                      name="yps")
                        for c in range(3):
                            nc.tensor.matmul(
                                yps[:, 0, :],
                                otq[:, c, qt4 * 128:(qt4 + 1) * 128],
                                wo_sb[:, c, 0:512],
                                start=(c == 0), stop=(c == 2),
                            )
                        for c in range(3):
                            nc.tensor.matmul(
                                yps[:, 1, 0:256],
                                otq[:, c, qt4 * 128:(qt4 + 1) * 128],
                                wo_sb[:, c, 512:768],
                                start=(c == 0), stop=(c == 2),
                            )
                        ysb = yout.tile([128, D], F32, tag="ysb")
                        if tail:
                            # split across engines — ACT is idle in the tail
                            nc.scalar.copy(ysb[:, 0:512], yps[:, 0, :])
                        else:
                            nc.vector.tensor_copy(ysb[:, 0:512], yps[:, 0, :])
                        nc.vector.tensor_copy(ysb[:, 512:768], yps[:, 1, 0:256])
                        nc.sync.dma_start(y_d[q0:q0 + 128, :], ysb[:, :])
                return fn

            # deferred Q work for tiles 4-15, as pieces (x is SBUF-resident)
            q_state = {}

            def q_piece_a(i):
                def fn():
                    psq = gemp.tile([128, 2, 512], F32, tag="gp",
                                    name=f"psq{i}")
                    for c in range(FC):
                        nc.tensor.matmul(
                            psq[:, 0, 0:384], xt_full[:, c, i * 128:(i + 1) * 128],
                            w_sbs[0][:, c, :],
                            start=(c == 0), stop=(c == FC - 1),
                        )
                    # DVE-path norm stats (ACT is saturated by exp)
                    q_state[i] = norm_rope(i, psq[:, 0:1, 0:384], 1, 0, "dve")
                return fn

            def q_piece_b(i):
                def fn():
                    ss, c2 = q_state.pop(i)
                    ph1_stage_b((i, 1, 0, ss, c2))
                return fn

            def flush_pv(p):
                qc, pp, ci, pt = p
                key = (qc, pp)
                if key not in cur_ots:
                    cur_ots[key] = [
                        otp.tile([128, 512], F32, tag="ot", name=f"ots{hh}")
                        for hh in range(2)
                    ]
                ots = cur_ots[key]
                for hh in range(2):
                    nc.tensor.matmul(
                        ots[hh][0:65, :],
                        vaug[:, ci, pp * 2 + hh, :],
                        pt[:, hh, :],
                        start=(ci == 0), stop=(ci == NCI - 1),
                    )
                if ci == NCI - 1:
                    if qc not in otun_by_qc:
                        otun_by_qc[qc] = otunp.tile(
                            [65, HPC, 512], BF16, tag="otun", name=f"otun{qc}")
                    otun_all = otun_by_qc[qc]
                    for hh in range(2):
                        nc.vector.tensor_copy(
                            otun_all[0:65, pp * 2 + hh, :], ots[hh][0:65, :])
                    del cur_ots[key]
                    if qc == 3:
                        # Last qc: den + normalize per pp so the final
                        # projection isn't one serial tail.
                        emit_den_tail(3, pp * 2, pp * 2 + 2)
                        norm_piece(3, pp * 2, pp * 2 + 2)()
                    elif pp == 2:
                        emit_den6(qc)
                elif ci == 4 and pp == 0 and qc > 0:
                    # qc-1's normalize/projection pieces: enqueue a few units
                    # into this qc (den6 DMA chain has landed by now) so they
                    # spread over ~44 piece slots instead of ~28.
                    finish_pieces.append(norm_piece(qc - 1, 0, 2))
                    finish_pieces.append(norm_piece(qc - 1, 2, 4))
                    finish_pieces.append(norm_piece(qc - 1, 4, 6))
                    for t in range(4):
                        finish_pieces.append(proj_piece(qc - 1, t, t + 1))

            unit_no = [0]

            def emit_unit(qc, pp, ci, pop_piece=True):
                slab = workp.tile([128, 2, 512], F32, tag="work", name="slab")
                for hh in range(2):
                    nc.tensor.matmul(
                        slab[:, hh, :],
                        kt_sb[hh * 64:(hh + 1) * 64, pp, ci * 128:(ci + 1) * 128],
                        qt_sb[hh * 64:(hh + 1) * 64, pp, qc * 512:(qc + 1) * 512],
                        start=True, stop=True,
                    )
                pt = ptp.tile([128, 2, 512], BF16, tag="pt")
                nc.scalar.activation(
                    pt[:, :, :], slab[:, :, :],
                    mybir.ActivationFunctionType.Exp,
                    bias=zero_t[:, :], scale=SCALE,
                )
                if pend[0] is not None:
                    flush_pv(pend[0])
                    if pop_piece and finish_pieces and unit_no[0] % 2 == 0:
                        finish_pieces.pop(0)()
                unit_no[0] += 1
                pend[0] = (qc, pp, ci, pt)

            # ================= schedule =================
            # Phase A: tiles 0-3 (q,k,v) — ACT path (exp stream not started)
            for i in range(NQ_EARLY):
                ps, psq = tile_gemm(i, True, gemp)
                flush_ph1()
                tile_chain(i, ps, psq, "act")

            # Phase B: tiles 4-15 (k,v) interleaved with (qc0, pp0) units.
            # Order per iteration: GEMM first (its slab-release wait overlaps
            # the previous iteration's work), stage_b flush, then units (their
            # kt/qt transposes are 2+ tiles old), then the tile's post chain.
            # kv slabs alternate between the two PSUM pools so production is
            # not single-buffered.
            emitted = 0
            for i in range(NQ_EARLY, NT):
                ps, _ = tile_gemm(i, False, workp if (i % 2) else gemp)
                flush_ph1()
                while emitted < max(0, i - 2):
                    emit_unit(0, 0, emitted, pop_piece=False)
                    emitted += 1
                tile_chain(i, ps, None, "mix")
            flush_ph1()

            # queue deferred q pieces (x resident — no reloads needed)
            for qi in range(NQ_EARLY, NT):
                finish_pieces.append(q_piece_a(qi))
                finish_pieces.append(q_piece_b(qi))

            # Phase C: remaining units
            for qc in range(NQC):
                for pp in range(3):
                    ci0 = emitted if (qc == 0 and pp == 0) else 0
                    emitted = -1
                    for ci in range(ci0, NCI):
                        emit_unit(qc, pp, ci)
            flush_pv(pend[0])
            for t in range(4):
                proj_piece(3, t, t + 1, workp if (t % 2) else gemp,
                           tail=True)()
            while finish_pieces:
                finish_pieces.pop(0)()

    split_multi_waits(nc)
    return nc


# ---------------------------------------------------------------- entry
def kernel(x, qkv_w, qkv_b, proj_w, proj_b, q_norm_w, k_norm_w, _trace=False,
           _debug=False):
    x = np.asarray(x, dtype=np.float32)
    qkv_w = np.asarray(qkv_w, dtype=np.float32)
    qkv_b = np.asarray(qkv_b, dtype=np.float32)
    proj_w = np.asarray(proj_w, dtype=np.float32)
    proj_b = np.asarray(proj_b, dtype=np.float32)
    q_norm_w = np.asarray(q_norm_w, dtype=np.float32)
    k_norm_w = np.asarray(k_norm_w, dtype=np.float32)

    use_bias = bool(np.any(qkv_b != 0.0))
    key = use_bias
    if key not in _CACHE:
        _CACHE[key] = build_nc(use_bias)
    nc = _CACHE[key]
    FC = 7 if use_bias else 6

    cosq, sinq = _rope_tables(q_norm_w)
    cosk, sink = _rope_tables(k_norm_w)
    cost = np.concatenate([cosq, cosk], axis=1)  # [N, 128]
    sint = np.concatenate([sinq, sink], axis=1)

    bf16 = ml_dtypes.bfloat16
    in_maps = []
    for core in range(8):
        b, hg = core // 2, core % 2
        h0 = hg * HPC
        cols = slice(h0 * HD, (h0 + HPC) * HD)
        xt = np.ascontiguousarray(x[b].T)                       # [768, N]
        wq = qkv_w[:, cols]
        wk = qkv_w[:, D:][:, cols]
        wv = qkv_w[:, 2 * D:][:, cols]
        if use_bias:
            pad = np.zeros((128, N), np.float32)
            pad[0, :] = 1.0
            xt = np.concatenate([xt, pad], axis=0)
            wpad = np.zeros((128, HPC * HD), np.float32)
            wqb = np.concatenate([wq, wpad], axis=0)
            wkb = np.concatenate([wk, wpad], axis=0)
            wvb = np.concatenate([wv, wpad], axis=0)
            wqb[D, :] = qkv_b[cols]
            wkb[D, :] = qkv_b[D:][cols]
            wvb[D, :] = qkv_b[2 * D:][cols]
            wq, wk, wv = wqb, wkb, wvb
        wo = proj_w[h0 * HD:(h0 + HPC) * HD, :]
        im = {
            "xt": xt.astype(bf16),
            "wq": np.ascontiguousarray(wq).astype(bf16),
            "wk": np.ascontiguousarray(wk).astype(bf16),
            "wv": np.ascontiguousarray(wv).astype(bf16),
            "wo": np.ascontiguousarray(wo).astype(bf16),
            "cost": cost.astype(bf16), "sint": sint.astype(bf16),
        }
        in_maps.append(im)

    res = run_bass_kernel_spmd(nc, in_maps, core_ids=list(range(8)),
                               trace=_trace or KERNEL_TRACE)
    kernel._last = res

    y = np.empty((B, N, D), dtype=np.float32)
    for b in range(B):
        y[b] = res.results[2 * b]["y"] + res.results[2 * b + 1]["y"] + proj_b[None, :]
    return y


# revision 36
# speedup vs baseline: 1.0248x; 1.0066x over previous
"""Multi-head attention (RMSNorm-QK + RoPE) Trainium2 Bass kernel — v6.

Sharding: 8 cores = 4 batches x 2 head-groups (6 heads each).
Host sums the two partial y's per batch and adds proj bias.

v6 design (vs the v2 baseline, 482us -> ~328us):
  - QK slabs shrink to 2 PSUM banks (2 fills/chunk, 192 units): frees 2
    banks for a dedicated piece pool (gemp), so deferred-q GEMMs /
    broadcast / projection matmuls no longer stall the QK->exp->PV
    rotation (was ~80us of exp-stream stalls).
  - QK fill pairs are (kt,hh0)+(kt,hh1): each pair runs concurrently in
    disjoint PE row groups (h0 / h64).
  - x is SBUF-resident (4 quarter-DMAs), weights load as one DMA each:
    ~5 DMA triggers before the first GEMM instead of ~25, and the
    deferred-q GEMMs need no HBM reloads.
  - Attention units for (qc0,pp0) interleave into phase-1 tile
    production (GEMM -> units -> chain order, kv slabs alternating
    between the two PSUM pools), so the exp stream starts at ~46us
    instead of ~108us.
  - Softmax denominators: rows gathered onto partitions 0..5 via tiny
    SBUF->SBUF DMAs (engines can't address base partitions outside
    {0,32,64,96}), 6-lane ACT ln/exp, reciprocal rows scattered back to
    partition 64 for the broadcast matmuls.  Last qc keeps the direct
    1-lane path (no DMA latency in the tail).
  - otun copies are bf16 (halves the flush copies and normalize muls).
  - Tail: per-pp den+normalize for the last qc, projection slabs
    alternate between both PSUM pools, output copies split ACT/DVE.

Steady state is ACT-exp-bound at ~1.14us per unit ([128,2,512] exp);
tensor ~80% busy (QK/PV/GEMM/proj + unhidden LDWEIGHTS).
"""

import sys

for _p in ("/opt/trn_rl_repo", "/root/.axon_site/_ro/trn_rl_repo"):
    if _p not in sys.path:
        sys.path.insert(0, _p)

import numpy as np
import ml_dtypes

import bass_rust
import concourse.bass as bass
import concourse.mybir as mybir
import concourse.tile as tile
from concourse.bass_utils import run_bass_kernel_spmd

# Problem constants (hardcoded per contract)
B, N, D = 4, 2048, 768
H, HD = 12, 64
HPC = 6              # heads per core
NT = N // 128        # 16 seq tiles
EPS = 1e-6
THETA = 10000.0
SCALE = HD ** -0.5   # 0.125

F32 = mybir.dt.float32
F32R = mybir.dt.float32r
BF16 = mybir.dt.bfloat16

KERNEL_TRACE = False
_CACHE = {}


# ---------------------------------------------------------------- wait split
_ctr = [0]


def _mk_nop(engine, waits=None, updates=None):
    _ctr[0] += 1
    si = mybir.SyncInfo(on_wait=waits or [], on_update=updates or [])
    return bass_rust.InstNoOp(
        name=f"I-waitfix-{_ctr[0]}", engine=engine, ins=[], outs=[], sync_info=si
    )


def split_multi_waits(nc):
    """This walrus build accepts only ONE sync wait/update per instruction;
    hoist extras onto adjacent same-engine NoOp carriers."""
    for fn in nc.m.functions:
        for bb in fn.blocks:
            insts = bb.instructions
            out = []
            changed = False
            for inst in insts:
                si = inst.sync_info
                if si is None:
                    out.append(inst)
                    continue
                waits = list(si.on_wait or [])
                updates = list(si.on_update or [])
                pre, post = [], []
                if len(waits) > 1:
                    for w in waits[:-1]:
                        pre.append(_mk_nop(inst.engine, waits=[w]))
                    si.on_wait = [waits[-1]]
                    changed = True
                if len(updates) > 1:
                    if inst.opcode == "DMACopy":
                        raise RuntimeError(
                            f"DMACopy {inst.name} has {len(updates)} updates"
                        )
                    for u in updates[1:]:
                        post.append(_mk_nop(inst.engine, updates=[u]))
                    si.on_update = [updates[0]]
                    changed = True
                out.extend(pre)
                out.append(inst)
                out.extend(post)
            if changed:
                insts[:] = out
    return nc


# ---------------------------------------------------------------- host utils
def _rope_tables(norm_w: np.ndarray):
    """cosw[n,d] = cos[n,d]*w[d];  sinw folds the rotate-half sign+swap of w:
    q' = qn*cosw + shuffle32(qn)*sinw  (shuffle32 = swap halves, no negation)."""
    inv_freq = 1.0 / (THETA ** (np.arange(0, HD, 2, dtype=np.float32) / HD))
    t = np.arange(N, dtype=np.float32)
    freqs = np.einsum("i,j->ij", t, inv_freq).astype(np.float32)
    emb = np.concatenate([freqs, freqs], axis=-1)  # [N, HD]
    cos = np.cos(emb).astype(np.float32)
    sin = np.sin(emb).astype(np.float32)
    w = norm_w.astype(np.float32)
    cosw = cos * w[None, :]
    sinw = np.empty_like(sin)
    h = HD // 2
    sinw[:, :h] = -sin[:, :h] * w[None, h:]
    sinw[:, h:] = sin[:, h:] * w[None, :h]
    return cosw, sinw


# ---------------------------------------------------------------- bass build
def build_nc(use_bias: bool):
    FC = 7 if use_bias else 6  # feature chunks of 128 (7th = bias row)
    nc = bass.Bass()

    xt_d = nc.dram_tensor("xt", [FC * 128, N], BF16, kind="ExternalInput")
    wq_d = nc.dram_tensor("wq", [FC * 128, HPC * HD], BF16, kind="ExternalInput")
    wk_d = nc.dram_tensor("wk", [FC * 128, HPC * HD], BF16, kind="ExternalInput")
    wv_d = nc.dram_tensor("wv", [FC * 128, HPC * HD], BF16, kind="ExternalInput")
    wo_d = nc.dram_tensor("wo", [HPC * HD, D], BF16, kind="ExternalInput")
    # rope tables: [:, 0, :] = q variant, [:, 1, :] = k variant (norm_w folded)
    cos_d = nc.dram_tensor("cost", [N, 2 * HD], BF16, kind="ExternalInput")
    sin_d = nc.dram_tensor("sint", [N, 2 * HD], BF16, kind="ExternalInput")
    y_d = nc.dram_tensor("y", [N, D], F32, kind="ExternalOutput")

    NQC = 4              # query chunks of 512
    NCI = NT             # 16 chunks of (kt, hh0)+(kt, hh1) per (qc, pp)

    with tile.TileContext(nc) as tc:
        with (
            tc.tile_pool(name="const", bufs=1) as constp,
            tc.tile_pool(name="wts", bufs=1) as wts,
            tc.tile_pool(name="persist", bufs=1) as persist,
            tc.tile_pool(name="rope", bufs=3) as rope,
            tc.tile_pool(name="ptp", bufs=4) as ptp,
            tc.tile_pool(name="otunp", bufs=2) as otunp,
            tc.tile_pool(name="otqp", bufs=2) as otqp,
            tc.tile_pool(name="denp", bufs=2) as denp,
            tc.tile_pool(name="yout", bufs=2) as yout,
            tc.tile_pool(name="work", bufs=2, space="PSUM") as workp,
            tc.tile_pool(name="gemp", bufs=1, space="PSUM") as gemp,
            tc.tile_pool(name="otp", bufs=2, space="PSUM") as otp,
        ):
            # ---- constants / weights
            ones_sb = constp.tile([128, 64], BF16)
            nc.vector.memset(ones_sb[:, :], 1.0)
            eps_t = constp.tile([128, 1], F32)
            nc.vector.memset(eps_t[:, :], EPS)
            zero_t = constp.tile([128, 1], F32)
            nc.vector.memset(zero_t[:, :], 0.0)

            # x stays SBUF-resident: 4 quarter-DMAs (one trigger each; each
            # spreads over all 16 hw queues).  First GEMM needs only quarter 0
            # plus wk/wv — ~5 triggers instead of 25.
            xt_full = persist.tile([128, FC, N], BF16, tag="xtf")
            xt_dr = xt_d.rearrange("(c p) n -> p c n", p=128)

            def load_xq(q):
                nc.sync.dma_start(
                    xt_full[:, :, q * 512:(q + 1) * 512],
                    xt_dr[:, :, q * 512:(q + 1) * 512],
                )

            load_xq(0)
            w_sbs = []
            for wd, nm in ((wk_d, "wk"), (wv_d, "wv"), (wq_d, "wq")):
                wsb = wts.tile([128, FC, HPC * HD], BF16, tag=nm, name=nm)
                nc.sync.dma_start(
                    wsb[:, :, :], wd.rearrange("(c p) n -> p c n", p=128))
                w_sbs.append(wsb)
            w_sbs = [w_sbs[2], w_sbs[0], w_sbs[1]]  # back to q, k, v order
            # rope tables (bf16 host-side)
            cos_sb = constp.tile([128, NT, 2, HD], BF16, tag="cos")
            nc.sync.dma_start(
                cos_sb[:, :, :, :],
                cos_d.rearrange("(t p) (a d) -> p t a d", p=128, a=2),
            )
            sin_sb = constp.tile([128, NT, 2, HD], BF16, tag="sin")
            nc.sync.dma_start(
                sin_sb[:, :, :, :],
                sin_d.rearrange("(t p) (a d) -> p t a d", p=128, a=2),
            )
            wo_sb = wts.tile([128, 3, D], BF16, tag="wo")
            nc.sync.dma_start(wo_sb[:, :, :], wo_d.rearrange("(c p) n -> p c n", p=128))
            for _q in range(1, 4):
                load_xq(_q)

            qt_sb = persist.tile([128, 3, N], BF16, tag="qt")
            kt_sb = persist.tile([128, 3, N], BF16, tag="kt")
            vaug = persist.tile([128, NT, HPC, 65], BF16, tag="vaug")
            nc.vector.memset(vaug[:, :, :, 64:65], 1.0)

            # ================= phase 1 helpers =================
            NQ_EARLY = 4

            def ph1_stage_b(p):
                # p = (i, na, v0, ss, c2); variants v0..v0+na-1 (0=q, 1=k)
                i, na, v0, ss, c2 = p
                lg = rope.tile([128, na, HPC], F32, tag="lg")
                nc.scalar.activation(lg[:, :, :], ss[:, :, :],
                                     mybir.ActivationFunctionType.Ln,
                                     bias=eps_t[:, :], scale=1.0 / HD)
                rs = rope.tile([128, na, HPC], BF16, tag="rs")
                nc.scalar.activation(rs[:, :, :], lg[:, :, :],
                                     mybir.ActivationFunctionType.Exp,
                                     bias=zero_t[:, :], scale=-0.5)
                ro = rope.tile([128, na, HPC, HD], BF16, tag="ro")
                nc.vector.tensor_mul(
                    ro[:, :, :, :], c2[:, :, :, :],
                    rs[:, :, :, None].to_broadcast((128, na, HPC, HD)),
                )
                rof = ro.rearrange("p a h d -> p (a h d)")
                for j in range(na):
                    dst = qt_sb if v0 + j == 0 else kt_sb
                    nc.sync.dma_start_transpose(
                        dst[:, 0:3, i * 128:(i + 1) * 128],
                        rof[:, j * 384:(j + 1) * 384])

            def norm_rope(i, ps_ap, na, v0, path):
                """ps_ap: PSUM [128, na, 384] raw q/k; returns (ss, c2).
                path: 'act' = copy+square on ACT, 'dve' = both on DVE,
                'mix' = copy on DVE + square on ACT."""
                src = ps_ap.rearrange("p a (h d) -> p a h d", h=HPC)
                qk = rope.tile([128, na, HPC, HD], BF16, tag="qk")
                sq = rope.tile([128, na, HPC, HD], BF16, tag="sq")
                if path == "act":
                    nc.scalar.copy(qk[:, :, :, :], src)
                    nc.scalar.activation(sq[:, :, :, :], src,
                                         mybir.ActivationFunctionType.Square,
                                         bias=zero_t[:, :])
                elif path == "mix":
                    nc.vector.tensor_copy(qk[:, :, :, :], src)
                    nc.scalar.activation(sq[:, :, :, :], src,
                                         mybir.ActivationFunctionType.Square,
                                         bias=zero_t[:, :])
                else:
                    nc.vector.tensor_copy(qk[:, :, :, :], src)
                    nc.vector.tensor_mul(sq[:, :, :, :], qk[:, :, :, :],
                                         qk[:, :, :, :])
                ss = rope.tile([128, na, HPC], F32, tag="ss")
                nc.vector.reduce_sum(ss[:, :, :], sq[:, :, :, :],
                                     axis=mybir.AxisListType.X)
                cosb = cos_sb[:, i, v0:v0 + na, None, :].to_broadcast(
                    (128, na, HPC, HD))
                sinb = sin_sb[:, i, v0:v0 + na, None, :]
                a = rope.tile([128, na, HPC, HD], BF16, tag="a")
                nc.vector.tensor_mul(a[:, :, :, :], qk[:, :, :, :], cosb)
                bt = rope.tile([128, na, HPC, HD], BF16, tag="bt")
                h = HD // 2
                nc.vector.tensor_mul(
                    bt[:, :, :, 0:h], qk[:, :, :, h:HD],
                    sinb[:, :, :, 0:h].to_broadcast((128, na, HPC, h)))
                nc.vector.tensor_mul(
                    bt[:, :, :, h:HD], qk[:, :, :, 0:h],
                    sinb[:, :, :, h:HD].to_broadcast((128, na, HPC, h)))
                c2 = rope.tile([128, na, HPC, HD], BF16, tag="c2")
                nc.vector.tensor_add(c2[:, :, :, :], a[:, :, :, :],
                                     bt[:, :, :, :])
                return ss, c2

            ph1_pend = []

            def flush_ph1():
                for pd in ph1_pend:
                    ph1_stage_b(pd)
                ph1_pend.clear()

            def tile_gemm(i, with_q, kv_pool):
                """K+V GEMM (+Q first, for early tiles) for seq tile i."""
                xs = xt_full[:, :, i * 128:(i + 1) * 128]
                psq = None
                if with_q:
                    # Q leads: qc0 units can't start until all four early q
                    # transposes land, so q's chain goes first everywhere.
                    psq = workp.tile([128, 2, 512], F32, tag="work",
                                     name=f"q{i}")
                    for c in range(FC):
                        nc.tensor.matmul(
                            psq[:, 0, 0:384], xs[:, c, :], w_sbs[0][:, c, :],
                            start=(c == 0), stop=(c == FC - 1),
                        )
                ps = kv_pool.tile([128, 2, 512], F32,
                                  tag="gp" if kv_pool is gemp else "work",
                                  name=f"kv{i}")
                for c in range(FC):
                    for t, wsb in enumerate((w_sbs[1], w_sbs[2])):
                        nc.tensor.matmul(
                            ps[:, t, 0:384], xs[:, c, :], wsb[:, c, :],
                            start=(c == 0), stop=(c == FC - 1),
                        )
                return ps, psq

            def tile_chain(i, ps, psq, path):
                pend_local = []
                if psq is not None:
                    ssq, c2q = norm_rope(i, psq[:, 0:1, 0:384], 1, 0, path)
                    pend_local.append((i, 1, 0, ssq, c2q))
                # V copy to vaug (DVE keeps ACT free for exp)
                nc.vector.tensor_copy(
                    vaug[:, i, :, 0:64],
                    ps[:, 1, 0:384].rearrange("p (h d) -> p h d", h=HPC),
                )
                ssk, c2k = norm_rope(i, ps[:, 0:1, 0:384], 1, 1, path)
                pend_local.append((i, 1, 1, ssk, c2k))
                ph1_pend.extend(pend_local)

            # ================= attention unit machinery =================
            cur_ots = {}
            otun_by_qc = {}
            den_by_qc = {}       # qc -> (den6 [6,512] f32, rec6 [6,512] bf16)
            rec_by_qc = {}       # qc -> rec64 [65, HPC, 512] bf16
            otq_by_qc = {}
            finish_pieces = []
            pend = [None]        # (qc, pp, ci, pt)

            def emit_den6(qc):
                # Gather the 6 den rows (partition 64 of otun) onto partitions
                # 0..5 via tiny SBUF->SBUF DMAs (engines can't address base
                # partitions outside {0,32,64,96}; DMA can), ln/exp 6-lane,
                # then scatter the reciprocal rows back to partition 64.
                otun_all = otun_by_qc[qc]
                den6i = denp.tile([6, 512], BF16, tag="den6i",
                                  name=f"den6i_{qc}")
                lnt = denp.tile([6, 512], F32, tag="lnt", name=f"lnt_{qc}")
                rec6 = denp.tile([6, 512], BF16, tag="rec6", name=f"rec6_{qc}")
                rec64 = rec_by_qc.setdefault(
                    qc, denp.tile([65, HPC, 512], BF16, tag="rec64",
                                  name=f"rec64_{qc}"))
                for hloc in range(HPC):
                    nc.sync.dma_start(den6i[hloc:hloc + 1, :],
                                      otun_all[64:65, hloc, :])
                nc.scalar.activation(lnt[0:6, :], den6i[0:6, :],
                                     mybir.ActivationFunctionType.Ln,
                                     bias=zero_t[0:6, :], scale=1.0)
                nc.scalar.activation(rec6[0:6, :], lnt[0:6, :],
                                     mybir.ActivationFunctionType.Exp,
                                     bias=zero_t[0:6, :], scale=-1.0)
                for hloc in range(HPC):
                    nc.sync.dma_start(rec64[64:65, hloc, :],
                                      rec6[hloc:hloc + 1, :])

            def emit_den_tail(qc, h0, h1):
                # Tail path (last qc): 1-lane ln/exp directly on the otun den
                # rows at partition 64 — no DMA latency in the critical tail.
                otun_all = otun_by_qc[qc]
                lg3 = denp.tile([65, 2, 512], F32, tag="lg3", name="lg3")
                rec64 = rec_by_qc.setdefault(
                    qc, denp.tile([65, HPC, 512], BF16, tag="rec64",
                                  name=f"rec64_{qc}"))
                nc.scalar.activation(lg3[64:65, :, :],
                                     otun_all[64:65, h0:h1, :],
                                     mybir.ActivationFunctionType.Ln,
                                     bias=zero_t[64:65, :], scale=1.0)
                nc.scalar.activation(rec64[64:65, h0:h1, :], lg3[64:65, :, :],
                                     mybir.ActivationFunctionType.Exp,
                                     bias=zero_t[64:65, :], scale=-1.0)

            def norm_piece(qc, h0, h1):
                def fn():
                    otun_all = otun_by_qc[qc]
                    rec64 = rec_by_qc[qc]
                    if qc not in otq_by_qc:
                        otq_by_qc[qc] = otqp.tile([128, 3, 512], BF16,
                                                  tag="otq", name=f"otq{qc}")
                    otq = otq_by_qc[qc]
                    bcw = gemp.tile([128, 2, 512], F32, tag="gp",
                                    name="bcw")
                    for j, hloc in enumerate(range(h0, h1)):
                        pp_, hh_ = hloc // 2, hloc % 2
                        nc.tensor.matmul(bcw[0:64, j, :],
                                         ones_sb[64:65, :],
                                         rec64[64:65, hloc, :],
                                         start=True, stop=True)
                        nc.vector.tensor_mul(
                            otq[hh_ * 64:(hh_ + 1) * 64, pp_, :],
                            otun_all[0:64, hloc, :],
                            bcw[0:64, j, :],
                        )
                return fn

            def proj_piece(qc, t0, t1, pool=None, tail=False):
                def fn():
                    pl = pool if pool is not None else gemp
                    otq = otq_by_qc[qc]
                    for qt4 in range(t0, t1):
                        q0 = qc * 512 + qt4 * 128
                        yps = pl.tile([128, 2, 512], F32,
                                      tag="gp" if pl is gemp else "work",
                                      name="yps")
                        for c in range(3):
                            nc.tensor.matmul(
                                yps[:, 0, :],
                                otq[:, c, qt4 * 128:(qt4 + 1) * 128],
                                wo_sb[:, c, 0:512],
                                start=(c == 0), stop=(c == 2),
                            )
                        for c in range(3):
                            nc.tensor.matmul(
                                yps[:, 1, 0:256],
                                otq[:, c, qt4 * 128:(qt4 + 1) * 128],
                                wo_sb[:, c, 512:768],
                                start=(c == 0), stop=(c == 2),
                            )
                        ysb = yout.tile([128, D], F32, tag="ysb")
                        if tail:
                            # split across engines — ACT is idle in the tail
                            nc.scalar.copy(ysb[:, 0:512], yps[:, 0, :])
                        else:
                            nc.vector.tensor_copy(ysb[:, 0:512], yps[:, 0, :])
                        nc.vector.tensor_copy(ysb[:, 512:768], yps[:, 1, 0:256])
                        nc.sync.dma_start(y_d[q0:q0 + 128, :], ysb[:, :])
                return fn

            # deferred Q work for tiles 4-15, as pieces (x is SBUF-resident)
            q_state = {}

            def q_piece_a(i):
                def fn():
                    psq = gemp.tile([128, 2, 512], F32, tag="gp",
                                    name=f"psq{i}")
                    for c in range(FC):
                        nc.tensor.matmul(
                            psq[:, 0, 0:384], xt_full[:, c, i * 128:(i + 1) * 128],
                            w_sbs[0][:, c, :],
                            start=(c == 0), stop=(c == FC - 1),
                        )
                    # DVE-path norm stats (ACT is saturated by exp)
                    q_state[i] = norm_rope(i, psq[:, 0:1, 0:384], 1, 0, "dve")
                return fn

            def q_piece_b(i):
                def fn():
                    ss, c2 = q_state.pop(i)
                    ph1_stage_b((i, 1, 0, ss, c2))
                return fn

            def flush_pv(p):
                qc, pp, ci, pt = p
                key = (qc, pp)
                if key not in cur_ots:
                    cur_ots[key] = [
                        otp.tile([128, 512], F32, tag="ot", name=f"ots{hh}")
                        for hh in range(2)
                    ]
                ots = cur_ots[key]
                for hh in range(2):
                    nc.tensor.matmul(
                        ots[hh][0:65, :],
                        vaug[:, ci, pp * 2 + hh, :],
                        pt[:, hh, :],
                        start=(ci == 0), stop=(ci == NCI - 1),
                    )
                if ci == NCI - 1:
                    if qc not in otun_by_qc:
                        otun_by_qc[qc] = otunp.tile(
                            [65, HPC, 512], BF16, tag="otun", name=f"otun{qc}")
                    otun_all = otun_by_qc[qc]
                    for hh in range(2):
                        nc.vector.tensor_copy(
                            otun_all[0:65, pp * 2 + hh, :], ots[hh][0:65, :])
                    del cur_ots[key]
                    if qc == 3:
                        # Last qc: den + normalize per pp so the final
                        # projection isn't one serial tail.
                        emit_den_tail(3, pp * 2, pp * 2 + 2)
                        norm_piece(3, pp * 2, pp * 2 + 2)()
                    elif pp == 2:
                        emit_den6(qc)
                elif ci == 4 and pp == 0 and qc > 0:
                    # qc-1's normalize/projection pieces: enqueue a few units
                    # into this qc (den6 DMA chain has landed by now) so they
                    # spread over ~44 piece slots instead of ~28.
                    finish_pieces.append(norm_piece(qc - 1, 0, 2))
                    finish_pieces.append(norm_piece(qc - 1, 2, 4))
                    finish_pieces.append(norm_piece(qc - 1, 4, 6))
                    for t in range(4):
                        finish_pieces.append(proj_piece(qc - 1, t, t + 1))

            unit_no = [0]

            def emit_unit(qc, pp, ci, pop_piece=True):
                slab = workp.tile([128, 2, 512], F32, tag="work", name="slab")
                for hh in range(2):
                    nc.tensor.matmul(
                        slab[:, hh, :],
                        kt_sb[hh * 64:(hh + 1) * 64, pp, ci * 128:(ci + 1) * 128],
                        qt_sb[hh * 64:(hh + 1) * 64, pp, qc * 512:(qc + 1) * 512],
                        start=True, stop=True,
                    )
                pt = ptp.tile([128, 2, 512], BF16, tag="pt")
                nc.scalar.activation(
                    pt[:, :, :], slab[:, :, :],
                    mybir.ActivationFunctionType.Exp,
                    bias=zero_t[:, :], scale=SCALE,
                )
                if pend[0] is not None:
                    flush_pv(pend[0])
                    if pop_piece and finish_pieces and unit_no[0] % 2 == 0:
                        finish_pieces.pop(0)()
                unit_no[0] += 1
                pend[0] = (qc, pp, ci, pt)

            # ================= schedule =================
            # Phase A: tiles 0-3 (q,k,v) — ACT path (exp stream not started)
            for i in range(NQ_EARLY):
                ps, psq = tile_gemm(i, True, gemp)
                flush_ph1()
                tile_chain(i, ps, psq, "act")

            # Phase B: tiles 4-15 (k,v) interleaved with (qc0, pp0) units.
            # Order per iteration: GEMM first (its slab-release wait overlaps
            # the previous iteration's work), stage_b flush, then units (their
            # kt/qt transposes are 2+ tiles old), then the tile's post chain.
            # kv slabs alternate between the two PSUM pools so production is
            # not single-buffered.
            emitted = 0
            for i in range(NQ_EARLY, NT):
                ps, _ = tile_gemm(i, False, workp if (i % 2) else gemp)
                flush_ph1()
                while emitted < max(0, i - 2):
                    emit_unit(0, 0, emitted, pop_piece=False)
                    emitted += 1
                tile_chain(i, ps, None, "mix")
            flush_ph1()

            # queue deferred q pieces (x resident — no reloads needed)
            for qi in range(NQ_EARLY, NT):
                finish_pieces.append(q_piece_a(qi))
                finish_pieces.append(q_piece_b(qi))

            # Phase C: remaining units
            for qc in range(NQC):
                for pp in range(3):
                    ci0 = emitted if (qc == 0 and pp == 0) else 0
                    emitted = -1
                    for ci in range(ci0, NCI):
                        emit_unit(qc, pp, ci)
            flush_pv(pend[0])
            for t in range(4):
                proj_piece(3, t, t + 1, workp if (t % 2) else gemp,
                           tail=True)()
            while finish_pieces:
                finish_pieces.pop(0)()

    split_multi_waits(nc)
    return nc


# ---------------------------------------------------------------- entry
def kernel(x, qkv_w, qkv_b, proj_w, proj_b, q_norm_w, k_norm_w, _trace=False,
           _debug=False):
    x = np.asarray(x, dtype=np.float32)
    qkv_w = np.asarray(qkv_w, dtype=np.float32)
    qkv_b = np.asarray(qkv_b, dtype=np.float32)
    proj_w = np.asarray(proj_w, dtype=np.float32)
    proj_b = np.asarray(proj_b, dtype=np.float32)
    q_norm_w = np.asarray(q_norm_w, dtype=np.float32)
    k_norm_w = np.asarray(k_norm_w, dtype=np.float32)

    use_bias = bool(np.any(qkv_b != 0.0))
    key = use_bias
    if key not in _CACHE:
        _CACHE[key] = build_nc(use_bias)
    nc = _CACHE[key]
    FC = 7 if use_bias else 6

    cosq, sinq = _rope_tables(q_norm_w)
    cosk, sink = _rope_tables(k_norm_w)
    cost = np.concatenate([cosq, cosk], axis=1)  # [N, 128]
    sint = np.concatenate([sinq, sink], axis=1)

    bf16 = ml_dtypes.bfloat16
    in_maps = []
    for core in range(8):
        b, hg = core // 2, core % 2
        h0 = hg * HPC
        cols = slice(h0 * HD, (h0 + HPC) * HD)
        xt = np.ascontiguousarray(x[b].T)                       # [768, N]
        wq = qkv_w[:, cols]
        wk = qkv_w[:, D:][:, cols]
        wv = qkv_w[:, 2 * D:][:, cols]
        if use_bias:
            pad = np.zeros((128, N), np.float32)
            pad[0, :] = 1.0
            xt = np.concatenate([xt, pad], axis=0)
            wpad = np.zeros((128, HPC * HD), np.float32)
            wqb = np.concatenate([wq, wpad], axis=0)
            wkb = np.concatenate([wk, wpad], axis=0)
            wvb = np.concatenate([wv, wpad], axis=0)
            wqb[D, :] = qkv_b[cols]
            wkb[D, :] = qkv_b[D:][cols]
            wvb[D, :] = qkv_b[2 * D:][cols]
            wq, wk, wv = wqb, wkb, wvb
        wo = proj_w[h0 * HD:(h0 + HPC) * HD, :]
        im = {
            "xt": xt.astype(bf16),
            "wq": np.ascontiguousarray(wq).astype(bf16),
            "wk": np.ascontiguousarray(wk).astype(bf16),
            "wv": np.ascontiguousarray(wv).astype(bf16),
            "wo": np.ascontiguousarray(wo).astype(bf16),
            "cost": cost.astype(bf16), "sint": sint.astype(bf16),
        }
        in_maps.append(im)

    res = run_bass_kernel_spmd(nc, in_maps, core_ids=list(range(8)),
                               trace=_trace or KERNEL_TRACE)
    kernel._last = res

    y = np.empty((B, N, D), dtype=np.float32)
    for b in range(B):
        y[b] = res.results[2 * b]["y"] + res.results[2 * b + 1]["y"] + proj_b[None, :]
    return y


# revision 40
# speedup vs baseline: 1.0270x; 1.0021x over previous
"""Multi-head attention (RMSNorm-QK + RoPE) Trainium2 Bass kernel — v6.

Sharding: 8 cores = 4 batches x 2 head-groups (6 heads each).
Host sums the two partial y's per batch and adds proj bias.

v6 design (vs the v2 baseline, 482us -> ~328us):
  - QK slabs shrink to 2 PSUM banks (2 fills/chunk, 192 units): frees 2
    banks for a dedicated piece pool (gemp), so deferred-q GEMMs /
    broadcast / projection matmuls no longer stall the QK->exp->PV
    rotation (was ~80us of exp-stream stalls).
  - QK fill pairs are (kt,hh0)+(kt,hh1): each pair runs concurrently in
    disjoint PE row groups (h0 / h64).
  - x is SBUF-resident (4 quarter-DMAs), weights load as one DMA each:
    ~5 DMA triggers before the first GEMM instead of ~25, and the
    deferred-q GEMMs need no HBM reloads.
  - Attention units for (qc0,pp0) interleave into phase-1 tile
    production (GEMM -> units -> chain order, kv slabs alternating
    between the two PSUM pools), so the exp stream starts at ~46us
    instead of ~108us.
  - Softmax denominators: rows gathered onto partitions 0..5 via tiny
    SBUF->SBUF DMAs (engines can't address base partitions outside
    {0,32,64,96}), 6-lane ACT ln/exp, reciprocal rows scattered back to
    partition 64 for the broadcast matmuls.  Last qc keeps the direct
    1-lane path (no DMA latency in the tail).
  - otun copies are bf16 (halves the flush copies and normalize muls).
  - Tail: per-pp den+normalize for the last qc, projection slabs
    alternate between both PSUM pools, output copies split ACT/DVE.

Steady state is ACT-exp-bound at ~1.14us per unit ([128,2,512] exp);
tensor ~80% busy (QK/PV/GEMM/proj + unhidden LDWEIGHTS).
"""

import sys

for _p in ("/opt/trn_rl_repo", "/root/.axon_site/_ro/trn_rl_repo"):
    if _p not in sys.path:
        sys.path.insert(0, _p)

import numpy as np
import ml_dtypes

import bass_rust
import concourse.bass as bass
import concourse.mybir as mybir
import concourse.tile as tile
from concourse.bass_utils import run_bass_kernel_spmd

# Problem constants (hardcoded per contract)
B, N, D = 4, 2048, 768
H, HD = 12, 64
HPC = 6              # heads per core
NT = N // 128        # 16 seq tiles
EPS = 1e-6
THETA = 10000.0
SCALE = HD ** -0.5   # 0.125

F32 = mybir.dt.float32
F32R = mybir.dt.float32r
BF16 = mybir.dt.bfloat16

KERNEL_TRACE = False
_CACHE = {}


# ---------------------------------------------------------------- wait split
_ctr = [0]


def _mk_nop(engine, waits=None, updates=None):
    _ctr[0] += 1
    si = mybir.SyncInfo(on_wait=waits or [], on_update=updates or [])
    return bass_rust.InstNoOp(
        name=f"I-waitfix-{_ctr[0]}", engine=engine, ins=[], outs=[], sync_info=si
    )


def split_multi_waits(nc):
    """This walrus build accepts only ONE sync wait/update per instruction;
    hoist extras onto adjacent same-engine NoOp carriers."""
    for fn in nc.m.functions:
        for bb in fn.blocks:
            insts = bb.instructions
            out = []
            changed = False
            for inst in insts:
                si = inst.sync_info
                if si is None:
                    out.append(inst)
                    continue
                waits = list(si.on_wait or [])
                updates = list(si.on_update or [])
                pre, post = [], []
                if len(waits) > 1:
                    for w in waits[:-1]:
                        pre.append(_mk_nop(inst.engine, waits=[w]))
                    si.on_wait = [waits[-1]]
                    changed = True
                if len(updates) > 1:
                    if inst.opcode == "DMACopy":
                        raise RuntimeError(
                            f"DMACopy {inst.name} has {len(updates)} updates"
                        )
                    for u in updates[1:]:
                        post.append(_mk_nop(inst.engine, updates=[u]))
                    si.on_update = [updates[0]]
                    changed = True
                out.extend(pre)
                out.append(inst)
                out.extend(post)
            if changed:
                insts[:] = out
    return nc


# ---------------------------------------------------------------- host utils
def _rope_tables(norm_w: np.ndarray):
    """cosw[n,d] = cos[n,d]*w[d];  sinw folds the rotate-half sign+swap of w:
    q' = qn*cosw + shuffle32(qn)*sinw  (shuffle32 = swap halves, no negation)."""
    inv_freq = 1.0 / (THETA ** (np.arange(0, HD, 2, dtype=np.float32) / HD))
    t = np.arange(N, dtype=np.float32)
    freqs = np.einsum("i,j->ij", t, inv_freq).astype(np.float32)
    emb = np.concatenate([freqs, freqs], axis=-1)  # [N, HD]
    cos = np.cos(emb).astype(np.float32)
    sin = np.sin(emb).astype(np.float32)
    w = norm_w.astype(np.float32)
    cosw = cos * w[None, :]
    sinw = np.empty_like(sin)
    h = HD // 2
    sinw[:, :h] = -sin[:, :h] * w[None, h:]
    sinw[:, h:] = sin[:, h:] * w[None, :h]
    return cosw, sinw


# ---------------------------------------------------------------- bass build
def build_nc(use_bias: bool):
    FC = 7 if use_bias else 6  # feature chunks of 128 (7th = bias row)
    nc = bass.Bass()

    xt_d = nc.dram_tensor("xt", [FC * 128, N], BF16, kind="ExternalInput")
    wq_d = nc.dram_tensor("wq", [FC * 128, HPC * HD], BF16, kind="ExternalInput")
    wk_d = nc.dram_tensor("wk", [FC * 128, HPC * HD], BF16, kind="ExternalInput")
    wv_d = nc.dram_tensor("wv", [FC * 128, HPC * HD], BF16, kind="ExternalInput")
    wo_d = nc.dram_tensor("wo", [HPC * HD, D], BF16, kind="ExternalInput")
    # rope tables: [:, 0, :] = q variant, [:, 1, :] = k variant (norm_w folded)
    cos_d = nc.dram_tensor("cost", [N, 2 * HD], BF16, kind="ExternalInput")
    sin_d = nc.dram_tensor("sint", [N, 2 * HD], BF16, kind="ExternalInput")
    y_d = nc.dram_tensor("y", [N, D], F32, kind="ExternalOutput")

    NQC = 4              # query chunks of 512
    NCI = NT             # 16 chunks of (kt, hh0)+(kt, hh1) per (qc, pp)

    with tile.TileContext(nc) as tc:
        with (
            tc.tile_pool(name="const", bufs=1) as constp,
            tc.tile_pool(name="wts", bufs=1) as wts,
            tc.tile_pool(name="persist", bufs=1) as persist,
            tc.tile_pool(name="rope", bufs=3) as rope,
            tc.tile_pool(name="ptp", bufs=4) as ptp,
            tc.tile_pool(name="otunp", bufs=2) as otunp,
            tc.tile_pool(name="otqp", bufs=2) as otqp,
            tc.tile_pool(name="denp", bufs=2) as denp,
            tc.tile_pool(name="yout", bufs=2) as yout,
            tc.tile_pool(name="work", bufs=2, space="PSUM") as workp,
            tc.tile_pool(name="gemp", bufs=1, space="PSUM") as gemp,
            tc.tile_pool(name="otp", bufs=2, space="PSUM") as otp,
        ):
            # ---- constants / weights
            ones_sb = constp.tile([128, 64], BF16)
            nc.vector.memset(ones_sb[:, :], 1.0)
            eps_t = constp.tile([128, 1], F32)
            nc.vector.memset(eps_t[:, :], EPS)
            zero_t = constp.tile([128, 1], F32)
            nc.vector.memset(zero_t[:, :], 0.0)

            # x stays SBUF-resident: 4 quarter-DMAs (one trigger each; each
            # spreads over all 16 hw queues).  First GEMM needs only quarter 0
            # plus wk/wv — ~5 triggers instead of 25.
            xt_full = persist.tile([128, FC, N], BF16, tag="xtf")
            xt_dr = xt_d.rearrange("(c p) n -> p c n", p=128)

            def load_xq(q):
                nc.sync.dma_start(
                    xt_full[:, :, q * 512:(q + 1) * 512],
                    xt_dr[:, :, q * 512:(q + 1) * 512],
                )

            nc.sync.dma_start(xt_full[:, :, 0:128], xt_dr[:, :, 0:128])
            w_sbs = []
            for wd, nm in ((wk_d, "wk"), (wv_d, "wv"), (wq_d, "wq")):
                wsb = wts.tile([128, FC, HPC * HD], BF16, tag=nm, name=nm)
                nc.sync.dma_start(
                    wsb[:, :, :], wd.rearrange("(c p) n -> p c n", p=128))
                w_sbs.append(wsb)
            w_sbs = [w_sbs[2], w_sbs[0], w_sbs[1]]  # back to q, k, v order
            # rope tables (bf16 host-side)
            cos_sb = constp.tile([128, NT, 2, HD], BF16, tag="cos")
            nc.sync.dma_start(
                cos_sb[:, :, :, :],
                cos_d.rearrange("(t p) (a d) -> p t a d", p=128, a=2),
            )
            sin_sb = constp.tile([128, NT, 2, HD], BF16, tag="sin")
            nc.sync.dma_start(
                sin_sb[:, :, :, :],
                sin_d.rearrange("(t p) (a d) -> p t a d", p=128, a=2),
            )
            wo_sb = wts.tile([128, 3, D], BF16, tag="wo")
            nc.sync.dma_start(wo_sb[:, :, :], wo_d.rearrange("(c p) n -> p c n", p=128))
            nc.sync.dma_start(xt_full[:, :, 128:512], xt_dr[:, :, 128:512])
            for _q in range(1, 4):
                load_xq(_q)

            qt_sb = persist.tile([128, 3, N], BF16, tag="qt")
            kt_sb = persist.tile([128, 3, N], BF16, tag="kt")
            vaug = persist.tile([128, NT, HPC, 65], BF16, tag="vaug")
            nc.vector.memset(vaug[:, :, :, 64:65], 1.0)

            # ================= phase 1 helpers =================
            NQ_EARLY = 4

            def ph1_stage_b(p):
                # p = (i, na, v0, ss, c2); variants v0..v0+na-1 (0=q, 1=k)
                i, na, v0, ss, c2 = p
                lg = rope.tile([128, 2, HPC], F32, tag="lg", name="lg")[:, 0:na]
                nc.scalar.activation(lg[:, :, :], ss[:, :, :],
                                     mybir.ActivationFunctionType.Ln,
                                     bias=eps_t[:, :], scale=1.0 / HD)
                rs = rope.tile([128, 2, HPC], BF16, tag="rs", name="rs")[:, 0:na]
                nc.scalar.activation(rs[:, :, :], lg[:, :, :],
                                     mybir.ActivationFunctionType.Exp,
                                     bias=zero_t[:, :], scale=-0.5)
                ro = rope.tile([128, 2, HPC, HD], BF16, tag="ro", name="ro")[:, 0:na]
                nc.vector.tensor_mul(
                    ro[:, :, :, :], c2[:, :, :, :],
                    rs[:, :, :, None].to_broadcast((128, na, HPC, HD)),
                )
                rof = ro.rearrange("p a h d -> p (a h d)")
                for j in range(na):
                    dst = qt_sb if v0 + j == 0 else kt_sb
                    nc.sync.dma_start_transpose(
                        dst[:, 0:3, i * 128:(i + 1) * 128],
                        rof[:, j * 384:(j + 1) * 384])

            def norm_rope(i, ps_ap, na, v0, path):
                """ps_ap: PSUM [128, na, 384] raw q/k; returns (ss, c2).
                path: 'act' = copy+square on ACT, 'dve' = both on DVE,
                'mix' = copy on DVE + square on ACT."""
                src = ps_ap.rearrange("p a (h d) -> p a h d", h=HPC)
                qk = rope.tile([128, 2, HPC, HD], BF16, tag="qk", name="qk")[:, 0:na]
                sq = rope.tile([128, 2, HPC, HD], BF16, tag="sq", name="sq")[:, 0:na]
                if path == "act":
                    nc.scalar.copy(qk[:, :, :, :], src)
                    nc.scalar.activation(sq[:, :, :, :], src,
                                         mybir.ActivationFunctionType.Square,
                                         bias=zero_t[:, :])
                elif path == "mix":
                    nc.vector.tensor_copy(qk[:, :, :, :], src)
                    nc.scalar.activation(sq[:, :, :, :], src,
                                         mybir.ActivationFunctionType.Square,
                                         bias=zero_t[:, :])
                else:
                    nc.vector.tensor_copy(qk[:, :, :, :], src)
                    nc.vector.tensor_mul(sq[:, :, :, :], qk[:, :, :, :],
                                         qk[:, :, :, :])
                ss = rope.tile([128, 2, HPC], F32, tag="ss", name="ss")[:, 0:na]
                nc.vector.reduce_sum(ss[:, :, :], sq[:, :, :, :],
                                     axis=mybir.AxisListType.X)
                cosb = cos_sb[:, i, v0:v0 + na, None, :].to_broadcast(
                    (128, na, HPC, HD))
                sinb = sin_sb[:, i, v0:v0 + na, None, :]
                a = rope.tile([128, 2, HPC, HD], BF16, tag="a", name="a")[:, 0:na]
                nc.vector.tensor_mul(a[:, :, :, :], qk[:, :, :, :], cosb)
                bt = rope.tile([128, 2, HPC, HD], BF16, tag="bt", name="bt")[:, 0:na]
                h = HD // 2
                nc.vector.tensor_mul(
                    bt[:, :, :, 0:h], qk[:, :, :, h:HD],
                    sinb[:, :, :, 0:h].to_broadcast((128, na, HPC, h)))
                nc.vector.tensor_mul(
                    bt[:, :, :, h:HD], qk[:, :, :, 0:h],
                    sinb[:, :, :, h:HD].to_broadcast((128, na, HPC, h)))
                c2 = rope.tile([128, 2, HPC, HD], BF16, tag="c2", name="c2")[:, 0:na]
                nc.vector.tensor_add(c2[:, :, :, :], a[:, :, :, :],
                                     bt[:, :, :, :])
                return ss, c2

            ph1_pend = []

            def flush_ph1():
                for pd in ph1_pend:
                    ph1_stage_b(pd)
                ph1_pend.clear()

            def tile_gemm(i, with_q, kv_pool):
                """Early tiles: (q,k) bands in one workp slab so the q+k
                norm/rope chain runs fused (na=2, half the op count), v alone
                in the kv_pool slab.  Later tiles: (k,v) bands as usual."""
                xs = xt_full[:, :, i * 128:(i + 1) * 128]
                psq = None
                if with_q:
                    psq = workp.tile([128, 2, 512], F32, tag="work",
                                     name=f"qk{i}")
                    for c in range(FC):
                        for t, wsb in enumerate((w_sbs[0], w_sbs[1])):
                            nc.tensor.matmul(
                                psq[:, t, 0:384], xs[:, c, :], wsb[:, c, :],
                                start=(c == 0), stop=(c == FC - 1),
                            )
                    ps = kv_pool.tile([128, 2, 512], F32,
                                      tag="gp" if kv_pool is gemp else "work",
                                      name=f"v{i}")
                    for c in range(FC):
                        nc.tensor.matmul(
                            ps[:, 0, 0:384], xs[:, c, :], w_sbs[2][:, c, :],
                            start=(c == 0), stop=(c == FC - 1),
                        )
                    return ps, psq
                ps = kv_pool.tile([128, 2, 512], F32,
                                  tag="gp" if kv_pool is gemp else "work",
                                  name=f"kv{i}")
                for c in range(FC):
                    for t, wsb in enumerate((w_sbs[1], w_sbs[2])):
                        nc.tensor.matmul(
                            ps[:, t, 0:384], xs[:, c, :], wsb[:, c, :],
                            start=(c == 0), stop=(c == FC - 1),
                        )
                return ps, psq

            def tile_chain(i, ps, psq, path):
                if psq is not None:
                    # fused q+k chain (variants 0=q, 1=k in psq bands 0,1)
                    nc.vector.tensor_copy(
                        vaug[:, i, :, 0:64],
                        ps[:, 0, 0:384].rearrange("p (h d) -> p h d", h=HPC),
                    )
                    ssqk, c2qk = norm_rope(i, psq[:, 0:2, 0:384], 2, 0, path)
                    ph1_pend.append((i, 2, 0, ssqk, c2qk))
                    return
                # V copy to vaug (DVE keeps ACT free for exp)
                nc.vector.tensor_copy(
                    vaug[:, i, :, 0:64],
                    ps[:, 1, 0:384].rearrange("p (h d) -> p h d", h=HPC),
                )
                ssk, c2k = norm_rope(i, ps[:, 0:1, 0:384], 1, 1, path)
                ph1_pend.append((i, 1, 1, ssk, c2k))

            # ================= attention unit machinery =================
            cur_ots = {}
            otun_by_qc = {}
            den_by_qc = {}       # qc -> (den6 [6,512] f32, rec6 [6,512] bf16)
            rec_by_qc = {}       # qc -> rec64 [65, HPC, 512] bf16
            otq_by_qc = {}
            finish_pieces = []
            tail_hold = []
            pend = [None]        # (qc, pp, ci, pt)

            def emit_den6(qc):
                # Gather the 6 den rows (partition 64 of otun) onto partitions
                # 0..5 via tiny SBUF->SBUF DMAs (engines can't address base
                # partitions outside {0,32,64,96}; DMA can), ln/exp 6-lane,
                # then scatter the reciprocal rows back to partition 64.
                otun_all = otun_by_qc[qc]
                den6i = denp.tile([6, 512], BF16, tag="den6i",
                                  name=f"den6i_{qc}")
                lnt = denp.tile([6, 512], F32, tag="lnt", name=f"lnt_{qc}")
                rec6 = denp.tile([6, 512], BF16, tag="rec6", name=f"rec6_{qc}")
                rec64 = rec_by_qc.setdefault(
                    qc, denp.tile([65, HPC, 512], BF16, tag="rec64",
                                  name=f"rec64_{qc}"))
                for hloc in range(HPC):
                    nc.sync.dma_start(den6i[hloc:hloc + 1, :],
                                      otun_all[64:65, hloc, :])
                nc.scalar.activation(lnt[0:6, :], den6i[0:6, :],
                                     mybir.ActivationFunctionType.Ln,
                                     bias=zero_t[0:6, :], scale=1.0)
                nc.scalar.activation(rec6[0:6, :], lnt[0:6, :],
                                     mybir.ActivationFunctionType.Exp,
                                     bias=zero_t[0:6, :], scale=-1.0)
                for hloc in range(HPC):
                    nc.sync.dma_start(rec64[64:65, hloc, :],
                                      rec6[hloc:hloc + 1, :])

            def emit_den_tail(qc, h0, h1):
                # Tail path (last qc): 1-lane ln/exp directly on the otun den
                # rows at partition 64 — no DMA latency in the critical tail.
                otun_all = otun_by_qc[qc]
                lg3 = denp.tile([65, 2, 512], F32, tag="lg3", name="lg3")
                rec64 = rec_by_qc.setdefault(
                    qc, denp.tile([65, HPC, 512], BF16, tag="rec64",
                                  name=f"rec64_{qc}"))
                nc.scalar.activation(lg3[64:65, :, :],
                                     otun_all[64:65, h0:h1, :],
                                     mybir.ActivationFunctionType.Ln,
                                     bias=zero_t[64:65, :], scale=1.0)
                nc.scalar.activation(rec64[64:65, h0:h1, :], lg3[64:65, :, :],
                                     mybir.ActivationFunctionType.Exp,
                                     bias=zero_t[64:65, :], scale=-1.0)

            def norm_piece(qc, h0, h1):
                def fn():
                    otun_all = otun_by_qc[qc]
                    rec64 = rec_by_qc[qc]
                    if qc not in otq_by_qc:
                        otq_by_qc[qc] = otqp.tile([128, 3, 512], BF16,
                                                  tag="otq", name=f"otq{qc}")
                    otq = otq_by_qc[qc]
                    bcw = gemp.tile([128, 2, 512], F32, tag="gp",
                                    name="bcw")
                    for j, hloc in enumerate(range(h0, h1)):
                        pp_, hh_ = hloc // 2, hloc % 2
                        nc.tensor.matmul(bcw[0:64, j, :],
                                         ones_sb[64:65, :],
                                         rec64[64:65, hloc, :],
                                         start=True, stop=True)
                        nc.vector.tensor_mul(
                            otq[hh_ * 64:(hh_ + 1) * 64, pp_, :],
                            otun_all[0:64, hloc, :],
                            bcw[0:64, j, :],
                        )
                return fn

            def proj_piece(qc, t0, t1, pool=None, tail=False):
                def fn():
                    pl = pool if pool is not None else gemp
                    otq = otq_by_qc[qc]
                    for qt4 in range(t0, t1):
                        q0 = qc * 512 + qt4 * 128
                        yps = pl.tile([128, 2, 512], F32,
                                      tag="gp" if pl is gemp else "work",
                                      name="yps")
                        for c in range(3):
                            nc.tensor.matmul(
                                yps[:, 0, :],
                                otq[:, c, qt4 * 128:(qt4 + 1) * 128],
                                wo_sb[:, c, 0:512],
                                start=(c == 0), stop=(c == 2),
                            )
                        for c in range(3):
                            nc.tensor.matmul(
                                yps[:, 1, 0:256],
                                otq[:, c, qt4 * 128:(qt4 + 1) * 128],
                                wo_sb[:, c, 512:768],
                                start=(c == 0), stop=(c == 2),
                            )
                        ysb = yout.tile([128, D], F32, tag="ysb")
                        if tail:
                            # split across engines — ACT is idle in the tail
                            nc.scalar.copy(ysb[:, 0:512], yps[:, 0, :])
                        else:
                            nc.vector.tensor_copy(ysb[:, 0:512], yps[:, 0, :])
                        nc.vector.tensor_copy(ysb[:, 512:768], yps[:, 1, 0:256])
                        nc.sync.dma_start(y_d[q0:q0 + 128, :], ysb[:, :])
                return fn

            # deferred Q work for tiles 4-15, as pieces (x is SBUF-resident)
            q_state = {}

            def q_piece_a(i):
                def fn():
                    psq = gemp.tile([128, 2, 512], F32, tag="gp",
                                    name=f"psq{i}")
                    for c in range(FC):
                        nc.tensor.matmul(
                            psq[:, 0, 0:384], xt_full[:, c, i * 128:(i + 1) * 128],
                            w_sbs[0][:, c, :],
                            start=(c == 0), stop=(c == FC - 1),
                        )
                    # DVE-path norm stats (ACT is saturated by exp)
                    q_state[i] = norm_rope(i, psq[:, 0:1, 0:384], 1, 0, "dve")
                return fn

            def q_piece_b(i):
                def fn():
                    ss, c2 = q_state.pop(i)
                    ph1_stage_b((i, 1, 0, ss, c2))
                return fn

            def flush_pv(p):
                qc, pp, ci, pt = p
                key = (qc, pp)
                if key not in cur_ots:
                    cur_ots[key] = [
                        otp.tile([128, 512], F32, tag="ot", name=f"ots{hh}")
                        for hh in range(2)
                    ]
                ots = cur_ots[key]
                for hh in range(2):
                    nc.tensor.matmul(
                        ots[hh][0:65, :],
                        vaug[:, ci, pp * 2 + hh, :],
                        pt[:, hh, :],
                        start=(ci == 0), stop=(ci == NCI - 1),
                    )
                if ci == NCI - 1:
                    if (qc, pp) == (3, 2):
                        for fn in tail_hold:
                            fn()
                        tail_hold.clear()
                    if qc not in otun_by_qc:
                        otun_by_qc[qc] = otunp.tile(
                            [65, HPC, 512], BF16, tag="otun", name=f"otun{qc}")
                    otun_all = otun_by_qc[qc]
                    for hh in range(2):
                        nc.vector.tensor_copy(
                            otun_all[0:65, pp * 2 + hh, :], ots[hh][0:65, :])
                    del cur_ots[key]
                    if qc == 3:
                        # Last qc: den + normalize per pp so the final
                        # projection isn't one serial tail.
                        emit_den_tail(3, pp * 2, pp * 2 + 2)
                        norm_piece(3, pp * 2, pp * 2 + 2)()
                    elif pp == 2:
                        emit_den6(qc)
                elif ci == 4 and pp == 0 and qc > 0:
                    # qc-1's normalize/projection pieces: enqueue a few units
                    # into this qc (den6 DMA chain has landed by now) so they
                    # spread over ~44 piece slots instead of ~28.
                    finish_pieces.append(norm_piece(qc - 1, 0, 2))
                    finish_pieces.append(norm_piece(qc - 1, 2, 4))
                    finish_pieces.append(norm_piece(qc - 1, 4, 6))
                    nproj = 2 if qc == 3 else 4
                    for t in range(nproj):
                        finish_pieces.append(proj_piece(qc - 1, t, t + 1))
                    if qc == 3:
                        # hold the last two qc2 projections for the tail: they
                        # keep the PE busy (and HAM warm) through the final
                        # den/normalize window.
                        tail_hold.append(proj_piece(2, 2, 3, workp))
                        tail_hold.append(proj_piece(2, 3, 4, workp))

            unit_no = [0]

            def emit_unit(qc, pp, ci, pop_piece=True):
                slab = workp.tile([128, 2, 512], F32, tag="work", name="slab")
                for hh in range(2):
                    nc.tensor.matmul(
                        slab[:, hh, :],
                        kt_sb[hh * 64:(hh + 1) * 64, pp, ci * 128:(ci + 1) * 128],
                        qt_sb[hh * 64:(hh + 1) * 64, pp, qc * 512:(qc + 1) * 512],
                        start=True, stop=True,
                    )
                pt = ptp.tile([128, 2, 512], BF16, tag="pt")
                nc.scalar.activation(
                    pt[:, :, :], slab[:, :, :],
                    mybir.ActivationFunctionType.Exp,
                    bias=zero_t[:, :], scale=SCALE,
                )
                if pend[0] is not None:
                    flush_pv(pend[0])
                    if pop_piece and finish_pieces and unit_no[0] % 2 == 0:
                        finish_pieces.pop(0)()
                unit_no[0] += 1
                pend[0] = (qc, pp, ci, pt)

            # ================= schedule =================
            # Phase A: tiles 0-3 (q,k,v) — ACT path (exp stream not started)
            for i in range(NQ_EARLY):
                ps, psq = tile_gemm(i, True, gemp)
                flush_ph1()
                tile_chain(i, ps, psq, "act")

            # Phase B: tiles 4-15 (k,v) interleaved with (qc0, pp0) units.
            # Order per iteration: GEMM first (its slab-release wait overlaps
            # the previous iteration's work), stage_b flush, then units (their
            # kt/qt transposes are 2+ tiles old), then the tile's post chain.
            # kv slabs alternate between the two PSUM pools so production is
            # not single-buffered.
            emitted = 0
            for i in range(NQ_EARLY, NT):
                ps, _ = tile_gemm(i, False, workp if (i % 2) else gemp)
                flush_ph1()
                while emitted < max(0, i - 2):
                    emit_unit(0, 0, emitted, pop_piece=False)
                    emitted += 1
                tile_chain(i, ps, None, "mix")
            flush_ph1()

            # queue deferred q pieces (x resident — no reloads needed)
            for qi in range(NQ_EARLY, NT):
                finish_pieces.append(q_piece_a(qi))
                finish_pieces.append(q_piece_b(qi))

            # Phase C: remaining units
            for qc in range(NQC):
                for pp in range(3):
                    ci0 = emitted if (qc == 0 and pp == 0) else 0
                    emitted = -1
                    for ci in range(ci0, NCI):
                        emit_unit(qc, pp, ci)
            flush_pv(pend[0])
            for t in range(4):
                proj_piece(3, t, t + 1, workp if (t % 2) else gemp,
                           tail=True)()
            while finish_pieces:
                finish_pieces.pop(0)()

    split_multi_waits(nc)
    return nc


# ---------------------------------------------------------------- entry
def kernel(x, qkv_w, qkv_b, proj_w, proj_b, q_norm_w, k_norm_w, _trace=False,
           _debug=False):
    x = np.asarray(x, dtype=np.float32)
    qkv_w = np.asarray(qkv_w, dtype=np.float32)
    qkv_b = np.asarray(qkv_b, dtype=np.float32)
    proj_w = np.asarray(proj_w, dtype=np.float32)
    proj_b = np.asarray(proj_b, dtype=np.float32)
    q_norm_w = np.asarray(q_norm_w, dtype=np.float32)
    k_norm_w = np.asarray(k_norm_w, dtype=np.float32)

    use_bias = bool(np.any(qkv_b != 0.0))
    key = use_bias
    if key not in _CACHE:
        _CACHE[key] = build_nc(use_bias)
    nc = _CACHE[key]
    FC = 7 if use_bias else 6

    cosq, sinq = _rope_tables(q_norm_w)
    cosk, sink = _rope_tables(k_norm_w)
    cost = np.concatenate([cosq, cosk], axis=1)  # [N, 128]
    sint = np.concatenate([sinq, sink], axis=1)

    bf16 = ml_dtypes.bfloat16
    in_maps = []
    for core in range(8):
        b, hg = core // 2, core % 2
        h0 = hg * HPC
        cols = slice(h0 * HD, (h0 + HPC) * HD)
        xt = np.ascontiguousarray(x[b].T)                       # [768, N]
        wq = qkv_w[:, cols]
        wk = qkv_w[:, D:][:, cols]
        wv = qkv_w[:, 2 * D:][:, cols]
        if use_bias:
            pad = np.zeros((128, N), np.float32)
            pad[0, :] = 1.0
            xt = np.concatenate([xt, pad], axis=0)
            wpad = np.zeros((128, HPC * HD), np.float32)
            wqb = np.concatenate([wq, wpad], axis=0)
            wkb = np.concatenate([wk, wpad], axis=0)
            wvb = np.concatenate([wv, wpad], axis=0)
            wqb[D, :] = qkv_b[cols]
            wkb[D, :] = qkv_b[D:][cols]
            wvb[D, :] = qkv_b[2 * D:][cols]
            wq, wk, wv = wqb, wkb, wvb
        wo = proj_w[h0 * HD:(h0 + HPC) * HD, :]
        im = {
            "xt": xt.astype(bf16),
            "wq": np.ascontiguousarray(wq).astype(bf16),
            "wk": np.ascontiguousarray(wk).astype(bf16),
            "wv": np.ascontiguousarray(wv).astype(bf16),
            "wo": np.ascontiguousarray(wo).astype(bf16),
            "cost": cost.astype(bf16), "sint": sint.astype(bf16),
        }
        in_maps.append(im)

    res = run_bass_kernel_spmd(nc, in_maps, core_ids=list(range(8)),
                               trace=_trace or KERNEL_TRACE)
    kernel._last = res

    y = np.empty((B, N, D), dtype=np.float32)
    for b in range(B):
        y[b] = res.results[2 * b]["y"] + res.results[2 * b + 1]["y"] + proj_b[None, :]
    return y
